# revision 34
# baseline (speedup 1.0000x reference)
"""DiffTransformerLayer on 8 trn2 NeuronCores.

Tensor-parallel attention: core c owns diff-head c (softmax heads 2c, 2c+1).
The rank-128 subln outputs o_fin are exchanged with one fp8 AllToAll per
batch (the b=0 exchange hides under b=1's attention); each core then owns a
contiguous 2x256-token slice (sl=c//2, half=c%2 of each batch) and applies
wo / the FFN locally.

Everything on-chip lives in transposed ("T") layout [feature, token]: scores
are computed transposed so softmax / LN / RMS reductions over features run as
ones-vector matmuls on the TensorEngine.  LN1/LN2 scale+bias are folded into
the projection weights on the host (w' = diag(ln_w) @ w, bias = ln_b @ w).

Perf structure vs the bf16 baseline:
- wo / w_in / w_out matmuls run in fp8 (e4m3) DoubleRow perf mode: 2x128
  contraction rows per pass at 0.5 cycles/row.  Weights are pre-scaled by 64
  (e4m3 normal range), o_fin by 8 (folded into subln), silu(g)*u by 8
  (folded into the up-bias); the scales divide back out in the epilogues.
- softmax denominators are accumulated on the (otherwise idle) GpSimd and
  Vector engines instead of per-tau ones-matmuls on the PE.
- exp() for both softmax heads of a tau runs as ONE scalar activation over a
  2-bank PSUM tile.
- FFN + wo weights are fully prefetched into SBUF during attention.
"""

import sys

if "/opt/trn_rl_repo" not in sys.path:
    sys.path.insert(0, "/opt/trn_rl_repo")

import numpy as np

import concourse.bacc as bacc
import concourse.bass as bass
import concourse.tile as tile
from concourse import mybir
from concourse import bass_utils

F32 = mybir.dt.float32
F32R = mybir.dt.float32r
BF16 = mybir.dt.bfloat16
F8 = mybir.dt.float8e4
NP_BF16 = mybir.dt.np(BF16)
NP_F8 = mybir.dt.np(F8)

B, S, D = 2, 2048, 1024
H = 8
HD = 64
DEPTH = 12
LAMBDA_INIT = float(0.8 - 0.6 * np.exp(-0.3 * (DEPTH - 1)))
FFN = 2 * D
N_CORES = 8
NS = B * S                  # 4096 flattened tokens
NT = NS // 128              # 32 token tiles
DK = D // 128               # 8 feature tiles
NSIG = NS // 512            # 8 sigma blocks
NI = FFN // 128             # 16 inner-dim tiles
EPS = 1e-5
SW = 64.0                   # fp8 weight pre-scale
SO = 8.0                    # fp8 o_fin pre-scale (folded into subln)
SU = 8.0                    # fp8 silu(g)*u pre-scale (folded into up-bias)
Exp = mybir.ActivationFunctionType.Exp
Sqrt = mybir.ActivationFunctionType.Sqrt
Silu = mybir.ActivationFunctionType.Silu
Ident = mybir.ActivationFunctionType.Identity
AluAdd = mybir.AluOpType.add
AluSub = mybir.AluOpType.subtract
AluMult = mybir.AluOpType.mult
DR = mybir.MatmulPerfMode.DoubleRow
RG = [list(range(N_CORES))]


def build_program(lam: float):
    nc = bacc.Bacc("TRN2", target_bir_lowering=False, debug=False,
                   enable_asserts=False, num_devices=N_CORES)

    # ---- external I/O (identical shapes on every core) ----
    x_nat = nc.dram_tensor("x_nat", [NS, D], BF16, kind="ExternalInput").ap()
    xT_own = nc.dram_tensor("xT_own", [D, 512], F32, kind="ExternalInput").ap()
    wq_s = nc.dram_tensor("wq_s", [D, 128], BF16, kind="ExternalInput").ap()
    wk_s = nc.dram_tensor("wk_s", [D, 128], BF16, kind="ExternalInput").ap()
    wv_s = nc.dram_tensor("wv_s", [D, 128], BF16, kind="ExternalInput").ap()
    # fp8 paired weights (DoubleRow layouts, already x SW)
    wo8_in = nc.dram_tensor("wo8", [128, DK // 2, 2, D], F8,
                            kind="ExternalInput").ap()
    wgu8_in = nc.dram_tensor("wgu8", [128, NI, 2, DK // 2, 2, 128], F8,
                             kind="ExternalInput").ap()
    wout8_in = nc.dram_tensor("wout8", [128, NI // 2, 2, D], F8,
                              kind="ExternalInput").ap()
    qb_in = nc.dram_tensor("qb", [128], F32, kind="ExternalInput").ap()
    kb_in = nc.dram_tensor("kb", [128], F32, kind="ExternalInput").ap()
    vb4_in = nc.dram_tensor("vb4", [512], BF16, kind="ExternalInput").ap()
    inb_in = nc.dram_tensor("inb", [2 * FFN], F32, kind="ExternalInput").ap()
    subln_eff = nc.dram_tensor("subln_eff", [128], F32, kind="ExternalInput").ap()
    masks_in = nc.dram_tensor("masks", [128, 4, 512], BF16, kind="ExternalInput").ap()
    ident_in = nc.dram_tensor("ident", [128, 128], BF16, kind="ExternalInput").ap()
    yT_out = nc.dram_tensor("yT", [D, 512], F32, kind="ExternalOutput").ap()
    import os
    DBG = os.environ.get("KDBG", "0") == "1"
    if DBG:
        dbg_out = nc.dram_tensor("dbg", [128, 12, 512], BF16,
                                 kind="ExternalOutput").ap()

    with tile.TileContext(nc) as tc:
        with (
            tc.tile_pool(name="persist", bufs=1) as persist,
            tc.tile_pool(name="ld", bufs=1) as ld,
            tc.tile_pool(name="stats", bufs=2) as stats,
            tc.tile_pool(name="dram", bufs=1, space="DRAM") as dram,
        ):
            # ---- constants / small inputs ----
            ones_c = persist.tile([128, 1], BF16, tag="ones_c")
            nc.vector.memset(ones_c, 1.0)
            rowinit = persist.tile([1, 128], F32, tag="rowinit")
            ones_rf = persist.tile([1, 128], F32R, tag="ones_rf")
            nc.vector.memset(rowinit, 1.0)
            with nc.allow_low_precision(reason="f32r constant rows"):
                nc.vector.tensor_copy(ones_rf, rowinit)
            ones_rb = persist.tile([1, 128], BF16, tag="ones_rb")
            nc.vector.memset(ones_rb, 1.0)
            rowinit2 = persist.tile([1, 128], F32, tag="rowinit2")
            lam_r = persist.tile([1, 128], F32R, tag="lam_r")
            nc.vector.memset(rowinit2, float(lam))
            with nc.allow_low_precision(reason="f32r constant rows"):
                nc.vector.tensor_copy(lam_r, rowinit2)
            eps128 = persist.tile([128, 1], F32, tag="eps128")
            nc.vector.memset(eps128, EPS)
            eps1 = persist.tile([1, 1], F32, tag="eps1")
            nc.vector.memset(eps1, EPS)
            subln_t = persist.tile([128, 1], F32, tag="subln")
            nc.sync.dma_start(out=subln_t,
                              in_=subln_eff.rearrange("(p one) -> p one", one=1))
            qb_t = persist.tile([128, 1], F32, tag="qb_t")
            nc.sync.dma_start(out=qb_t, in_=qb_in.rearrange("(p one) -> p one", one=1))
            kb_t = persist.tile([128, 1], F32, tag="kb_t")
            nc.sync.dma_start(out=kb_t, in_=kb_in.rearrange("(p one) -> p one", one=1))
            vb4_r = persist.tile([1, 512], BF16, tag="vb4_r")
            nc.sync.dma_start(out=vb4_r, in_=vb4_in.rearrange("(one f) -> one f", one=1))
            inb_t = persist.tile([128, 2 * NI], F32, tag="inb_t")
            nc.sync.dma_start(out=inb_t, in_=inb_in.rearrange("(k p) -> p k", p=128))

            # ---- fp8 weights + A2A landing (D/E lifetime; DMAs issued post-B) ----
            pW_cm = tc.tile_pool(name="pW", bufs=1)
            pW = pW_cm.__enter__()
            wo8 = pW.tile([128, DK // 2, 2, D], F8, tag="wo8")
            wgu8 = pW.tile([128, NI, 2, DK // 2, 2, 128], F8, tag="wgu8")
            wout8 = pW.tile([128, NI // 2, 2, D], F8, tag="wout8")
            af = [pW.tile([128, DK, 4, 64], F8, tag=f"af{b}", name=f"af{b}")
                  for b in range(B)]

            # ---- stage A/B/C lifetime pool ----
            pqkv_cm = tc.tile_pool(name="pqkv", bufs=1)
            pqkv = pqkv_cm.__enter__()
            qT = [pqkv.tile([128, 512], BF16, tag=f"qT{s}", name=f"qT{s}")
                  for s in range(NSIG)]
            kT = [pqkv.tile([128, 512], BF16, tag=f"kT{s}", name=f"kT{s}")
                  for s in range(NSIG)]
            v_t = [pqkv.tile([128, 512], BF16, tag=f"v{s}", name=f"v{s}")
                   for s in range(NSIG)]
            ident = pqkv.tile([128, 128], BF16, tag="ident")
            nc.sync.dma_start(out=ident, in_=ident_in)
            masks = pqkv.tile([128, 4, 512], BF16, tag="masks")
            nc.sync.dma_start(out=masks, in_=masks_in)
            wq_sb = pqkv.tile([128, D], BF16, tag="wq_sb")
            wk_sb = pqkv.tile([128, D], BF16, tag="wk_sb")
            wv_sb = pqkv.tile([128, D], BF16, tag="wv_sb")
            # qkv weights land before the bulk x stream
            for sb_t, wsrc in ((wq_sb, wq_s), (wk_sb, wk_s), (wv_sb, wv_s)):
                nc.sync.dma_start(
                    out=sb_t.rearrange("p (k m) -> p k m", m=128),
                    in_=wsrc.rearrange("(k p) m -> p k m", p=128))

            # AllToAll bounce buffers, one per (b, sl) sigma block: chunk u of
            # the input is o_fin[:, 64u:64u+64]; after the exchange, out[h] is
            # head h's o_fin for OUR 64-token unit of that sigma block.
            a2a_in = [dram.tile([N_CORES, 128, 64], F8, tag=f"a2ai{s}",
                                name=f"a2ai{s}") for s in range(NSIG)]
            a2a_out = [dram.tile([N_CORES, 128, 64], F8, tag=f"a2ao{s}",
                                 name=f"a2ao{s}") for s in range(NSIG)]

            # shared PSUM pool for stages A+B+C (8 banks exactly)
            psC_cm = tc.tile_pool(name="psC", bufs=1, space="PSUM")
            psC = psC_cm.__enter__()

            # v bias broadcast [128, 512] (4 repeats of the 128-wide bias row)
            pbv = psC.tile([128, 512], F32, tag="o1")
            nc.tensor.matmul(pbv, lhsT=ones_rb, rhs=vb4_r, start=True, stop=True)
            bv_bc = pqkv.tile([128, 512], F32, tag="bv_bc")
            nc.vector.tensor_copy(bv_bc, pbv)

            # ================= Stage A: LN1 + transpose =================
            # hT is a rotating per-sigma pipeline: stage B only reads the
            # tiles of its own sigma block, so 3 bufs per group suffice.
            phT_cm = tc.tile_pool(name="phT", bufs=1)
            phT = phT_cm.__enter__()
            hTs = []
            for s8 in range(NSIG):
                hTg = [phT.tile([128, 4, 512], BF16, tag=f"hT{g}", bufs=3,
                                name=f"hT{g}_{s8}") for g in range(2)]
                hTs.append(hTg)
                mvg = stats.tile([128, 4, 2], F32, tag="mvg")
                x_t4 = ld.tile([128, 4, D], BF16, tag="x_t", bufs=2)
                nc.sync.dma_start(
                    out=x_t4,
                    in_=x_nat[s8 * 512:(s8 + 1) * 512, :]
                    .rearrange("(j p) d -> p j d", p=128))
                for j4 in range(4):
                    st_t = stats.tile([128, 2, 6], F32, tag="bst")
                    xg = x_t4[:, j4, :].rearrange("p (g d) -> p g d", g=2)
                    for g in range(2):
                        nc.vector.bn_stats(out=st_t[:, g, :], in_=xg[:, g, :])
                    nc.vector.bn_aggr(out=mvg[:, j4, :], in_=st_t)
                rstd4 = stats.tile([128, 4], F32, tag="rstd4")
                nc.scalar.activation(out=rstd4, in_=mvg[:, :, 1], func=Sqrt,
                                     bias=eps128, scale=1.0)
                nc.vector.reciprocal(out=rstd4, in_=rstd4)
                for j4 in range(4):
                    st = s8 * 4 + j4
                    h_t = ld.tile([128, D], BF16, tag="h_t", bufs=4)
                    nc.vector.tensor_scalar(out=h_t, in0=x_t4[:, j4, :],
                                            scalar1=mvg[:, j4, 0:1],
                                            scalar2=rstd4[:, j4:j4 + 1],
                                            op0=AluSub, op1=AluMult)
                    jcol = slice(j4 * 128, (j4 + 1) * 128)
                    for g4 in range(2):
                        tpw = psC.tile([128, 1024], BF16, tag="tp", bufs=2, name="tp")
                        tp = tpw[:, 0:512]
                        for j in range(4):
                            dk = g4 * 4 + j
                            nc.tensor.transpose(tp[:, j * 128:(j + 1) * 128],
                                                h_t[:, dk * 128:(dk + 1) * 128], ident)
                        dst = hTs[s8][g4][:, :, jcol]
                        srcv = tp.rearrange("p (j f) -> p j f", f=128)
                        if (st + g4) % 2 == 0:
                            nc.vector.tensor_copy(dst, srcv)
                        else:
                            nc.scalar.copy(dst, srcv)

            # ================= Stage B: q,k,v projections =================
            for sg in range(NSIG):
                psq = psC.tile([128, 512], F32, tag="s12", bufs=2)
                for kk in range(DK):
                    nc.tensor.matmul(psq, lhsT=wq_sb[:, kk * 128:(kk + 1) * 128],
                                     rhs=hTs[sg][kk // 4][:, kk % 4, :],
                                     start=(kk == 0), stop=(kk == DK - 1))
                nc.scalar.activation(out=qT[sg], in_=psq, func=Ident,
                                     scale=1.0, bias=qb_t)
                psk = psC.tile([128, 512], F32, tag="s12", bufs=2)
                for kk in range(DK):
                    nc.tensor.matmul(psk, lhsT=wk_sb[:, kk * 128:(kk + 1) * 128],
                                     rhs=hTs[sg][kk // 4][:, kk % 4, :],
                                     start=(kk == 0), stop=(kk == DK - 1))
                nc.scalar.activation(out=kT[sg], in_=psk, func=Ident,
                                     scale=1.0, bias=kb_t)
                psv = psC.tile([128, 512], F32, tag="s12", bufs=2)
                for j4 in range(4):
                    for kk in range(DK):
                        nc.tensor.matmul(psv[:, j4 * 128:(j4 + 1) * 128],
                                         lhsT=hTs[sg][kk // 4][:, kk % 4, j4 * 128:(j4 + 1) * 128],
                                         rhs=wv_sb[:, kk * 128:(kk + 1) * 128],
                                         start=(kk == 0), stop=(kk == DK - 1))
                nc.vector.tensor_add(v_t[sg], psv, bv_bc)
            phT_cm.__exit__(None, None, None)

            # ---- weight prefetch (overlaps attention) ----
            nc.sync.dma_start(out=wo8, in_=wo8_in)
            nc.sync.dma_start(out=wgu8, in_=wgu8_in)
            nc.sync.dma_start(out=wout8, in_=wout8_in)

            # ================= Stage C: differential attention =================
            pwc_cm = tc.tile_pool(name="pwc", bufs=1)
            pwc = pwc_cm.__enter__()
            for b in range(B):
                for sl in (0, 1, 2, 3):
                    sg = 4 * b + sl
                    ntau = 4 * (sl + 1)
                    o1 = psC.tile([128, 512], F32, tag="o1")
                    o2 = psC.tile([128, 512], F32, tag="o2")
                    esum1 = pwc.tile([128, 512], BF16, tag="es1", bufs=2)
                    esum2 = pwc.tile([128, 512], BF16, tag="es2", bufs=2)
                    for tau in range(ntau):
                        tg = 16 * b + tau
                        ts8, tj = tg // 4, tg % 4
                        tcol = slice(tj * 128, (tj + 1) * 128)
                        rel = tau - 4 * sl
                        off = max(rel, 0) * 128          # causal column offset
                        ecol = slice(off, 512)
                        st_fl = (tau == 0)
                        sp_fl = (tau == ntau - 1)
                        s12 = psC.tile([128, 2, 512], F32, tag="s12", bufs=2)
                        nc.tensor.matmul(s12[:, 0, ecol], lhsT=kT[ts8][0:64, tcol],
                                         rhs=qT[sg][0:64, ecol], start=True, stop=True)
                        nc.tensor.matmul(s12[:, 1, ecol], lhsT=kT[ts8][64:128, tcol],
                                         rhs=qT[sg][64:128, ecol], start=True, stop=True)
                        e12 = pwc.tile([128, 2, 512], BF16, tag="e12", bufs=6)
                        nc.scalar.activation(out=e12[:, :, ecol], in_=s12[:, :, ecol],
                                             func=Exp)
                        e1 = e12[:, 0, :]
                        e2 = e12[:, 1, :]
                        if rel >= 0:
                            # only the 128-wide diagonal strip needs masking
                            strip = slice(off, off + 128)
                            tri = masks[:, 0, 0:128]
                            nc.gpsimd.tensor_mul(e1[:, strip], e1[:, strip], tri)
                            nc.vector.tensor_mul(e2[:, strip], e2[:, strip], tri)
                        if st_fl:
                            nc.gpsimd.tensor_copy(esum1, e1)
                            nc.vector.tensor_copy(esum2, e2)
                        else:
                            nc.gpsimd.tensor_add(esum1[:, ecol], e1[:, ecol],
                                                 esum1[:, ecol])
                            nc.vector.tensor_add(esum2[:, ecol], e2[:, ecol],
                                                 esum2[:, ecol])
                        nc.tensor.matmul(o1[:, ecol], lhsT=v_t[ts8][:, tcol],
                                         rhs=e1[:, ecol], start=st_fl, stop=sp_fl)
                        nc.tensor.matmul(o2[:, ecol], lhsT=v_t[ts8][:, tcol],
                                         rhs=e2[:, ecol], start=st_fl, stop=sp_fl)
                    # ---- differential combine + subln ----
                    z1 = psC.tile([1, 512], F32, tag="tp", bufs=2)
                    nc.tensor.matmul(z1, lhsT=ones_c, rhs=esum1, start=True, stop=True)
                    z2 = psC.tile([1, 512], F32, tag="tp", bufs=2)
                    nc.tensor.matmul(z2, lhsT=ones_c, rhs=esum2, start=True, stop=True)
                    zrec = stats.tile([1, 512], F32, tag="zrec")
                    nc.vector.reciprocal_approx_fast(out=zrec, in_=z2)
                    zr = stats.tile([1, 512], F32R, tag="rowf1")
                    with nc.allow_low_precision(reason="softmax ratio to f32r row"):
                        nc.vector.tensor_mul(zr, z1, zrec)
                    w_bc = psC.tile([128, 512], F32, tag="tp", bufs=2)
                    nc.tensor.matmul(w_bc, lhsT=lam_r, rhs=zr, start=True, stop=True)
                    w_sb = pwc.tile([128, 512], F32, tag="w_sb")
                    nc.vector.tensor_copy(w_sb, w_bc)
                    t_sb = pwc.tile([128, 512], F32, tag="t_sb")
                    nc.vector.tensor_mul(t_sb, o2, w_sb)
                    oc = pwc.tile([128, 512], F32, tag="oc")
                    nc.vector.tensor_sub(oc, o1, t_sb)
                    sq = pwc.tile([128, 512], BF16, tag="sq")
                    nc.gpsimd.tensor_mul(sq, oc, oc)
                    ss = psC.tile([1, 512], F32, tag="tp", bufs=2)
                    nc.tensor.matmul(ss, lhsT=ones_c, rhs=sq, start=True, stop=True)
                    rt = stats.tile([1, 512], F32, tag="rt")
                    nc.scalar.activation(out=rt, in_=ss, func=Sqrt,
                                         scale=1.0 / 128.0, bias=eps1)
                    rrf = stats.tile([1, 512], F32, tag="rowf3")
                    nc.vector.reciprocal_approx_fast(out=rrf, in_=rt)
                    rr = stats.tile([1, 512], F32R, tag="rowf2")
                    with nc.allow_low_precision(reason="rms recip to f32r row"):
                        nc.vector.tensor_copy(rr, rrf)
                    r_bc = psC.tile([128, 512], F32, tag="tp", bufs=2)
                    nc.tensor.matmul(r_bc, lhsT=ones_rf, rhs=rr, start=True, stop=True)
                    t2 = pwc.tile([128, 512], F32, tag="t2")
                    nc.vector.tensor_mul(t2, oc, r_bc)
                    o_fin = pwc.tile([128, 512], F8, tag="o_fin", bufs=4)
                    nc.vector.tensor_scalar_mul(o_fin, t2, subln_t)
                    nc.sync.dma_start(
                        out=a2a_in[sg].rearrange("u p f -> p u f"),
                        in_=o_fin.rearrange("p (u f) -> p u f", f=64))
                    # tiny per-sigma AllToAll: all but the last overlap the
                    # remaining attention compute
                    nc.gpsimd.collective_compute(
                        "AllToAll", mybir.AluOpType.bypass, replica_groups=RG,
                        ins=[a2a_in[sg].opt()], outs=[a2a_out[sg].opt()])
                if DBG and b == 0:
                    dv = dbg_out.rearrange("p s f -> p (s f)")
                    nc.sync.dma_start(
                        out=dv[:, 0:2048].rearrange("p (u f) -> u p f", f=256),
                        in_=a2a_in[0])
                    nc.sync.dma_start(
                        out=dv[:, 2048:4096].rearrange("p (u f) -> u p f", f=256),
                        in_=a2a_out[0])
                    nc.sync.dma_start(
                        out=dv[:, 4096:6144].rearrange("p (u f) -> p u f", f=256),
                        in_=af[0])
            pwc_cm.__exit__(None, None, None)
            psC_cm.__exit__(None, None, None)
            pqkv_cm.__exit__(None, None, None)

            # ================= Stage D: local wo + residuals =================
            pE_cm = tc.tile_pool(name="pE", bufs=1)
            pE = pE_cm.__enter__()
            psE_cm = tc.tile_pool(name="psE", bufs=1, space="PSUM")
            psE = psE_cm.__enter__()
            for b in range(B):
                for sl in range(4):
                    nc.sync.dma_start(
                        out=af[b][:, :, sl, :],
                        in_=a2a_out[4 * b + sl].rearrange("h p f -> p h f"))
            y1own = [pE.tile([128, 512], F32, tag=f"y1own{dk}", name=f"y1own{dk}")
                     for dk in range(DK)]
            y1bf = [pE.tile([128, 512], BF16, tag=f"y1bf{dk}", name=f"y1bf{dk}")
                    for dk in range(DK)]
            for dm in range(DK):
                xo_t = ld.tile([128, 512], F32, tag="xo_t", bufs=2)
                nc.sync.dma_start(out=xo_t, in_=xT_own[dm * 128:(dm + 1) * 128, :])
                dmc = slice(dm * 128, (dm + 1) * 128)
                for b in range(B):
                    hcol = slice(b * 256, (b + 1) * 256)
                    pwo = psE.tile([128, 256], F32, tag="ey2", bufs=2)
                    afp = af[b].rearrange("p (h2 i) s f -> p h2 i (s f)", i=2)
                    for h2 in range(DK // 2):
                        nc.tensor.matmul(pwo, lhsT=wo8[:, h2, :, dmc],
                                         rhs=afp[:, h2], start=(h2 == 0),
                                         stop=(h2 == DK // 2 - 1), perf_mode=DR)
                    nc.vector.scalar_tensor_tensor(
                        out=y1own[dm][:, hcol], in0=pwo, scalar=1.0 / (SW * SO),
                        in1=xo_t[:, hcol], op0=AluMult, op1=AluAdd)
                    with nc.allow_low_precision(reason="ffn input is bf16"):
                        nc.vector.scalar_tensor_tensor(
                            out=y1bf[dm][:, hcol], in0=pwo, scalar=1.0 / (SW * SO),
                            in1=xo_t[:, hcol], op0=AluMult, op1=AluAdd)

            # ================= Stage E: LN2 + FFN (local) =================
            ssum = psE.tile([1, 512], F32, tag="es")
            ssq = psE.tile([1, 512], F32, tag="esq")
            for dk in range(DK):
                nc.tensor.matmul(ssum, lhsT=ones_c, rhs=y1bf[dk],
                                 start=(dk == 0), stop=(dk == DK - 1))
                sqt = ld.tile([128, 512], BF16, tag="sqt", bufs=2)
                nc.vector.tensor_mul(sqt, y1bf[dk], y1bf[dk])
                nc.tensor.matmul(ssq, lhsT=ones_c, rhs=sqt,
                                 start=(dk == 0), stop=(dk == DK - 1))
            m_row = stats.tile([1, 512], F32, tag="rowf1")
            nc.vector.tensor_scalar_mul(m_row, ssum, 1.0 / float(D))
            mm_row = stats.tile([1, 512], F32, tag="rowf2")
            nc.vector.tensor_mul(mm_row, m_row, m_row)
            v_row = stats.tile([1, 512], F32, tag="rowf3")
            nc.vector.tensor_scalar_mul(v_row, ssq, 1.0 / float(D))
            nc.vector.tensor_sub(v_row, v_row, mm_row)
            nc.scalar.activation(out=v_row, in_=v_row, func=Sqrt,
                                 scale=1.0, bias=eps1)
            r_row = stats.tile([1, 512], F32R, tag="rowf4")
            mr_row = stats.tile([1, 512], F32R, tag="rowf5")
            with nc.allow_low_precision(reason="ln2 rows to f32r"):
                nc.vector.reciprocal(out=r_row, in_=v_row)
                nc.vector.tensor_mul(mr_row, m_row, r_row)
            pbc = psE.tile([128, 512], F32, tag="es")
            nc.tensor.matmul(pbc, lhsT=ones_rf, rhs=r_row, start=True, stop=True)
            rbc2 = pE.tile([128, 512], BF16, tag="rbc2")
            nc.vector.tensor_copy(rbc2, pbc)
            pbc2 = psE.tile([128, 512], F32, tag="esq")
            nc.tensor.matmul(pbc2, lhsT=ones_rf, rhs=mr_row, start=True, stop=True)
            mrbc = pE.tile([128, 512], BF16, tag="mrbc")
            nc.vector.tensor_copy(mrbc, pbc2)
            # h2 in fp8, paired along the contraction dim for DoubleRow
            h2p = [pE.tile([128, 2, 512], F8, tag=f"h2p{k}", name=f"h2p{k}")
                   for k in range(DK // 2)]
            for dk in range(DK):
                a = ld.tile([128, 512], BF16, tag="h2t", bufs=2)
                nc.vector.tensor_mul(a, y1bf[dk], rbc2)
                nc.gpsimd.tensor_sub(h2p[dk // 2][:, dk % 2, :], a, mrbc)
            su_p = [pE.tile([128, 2, 512], F8, tag=f"sup{k}", name=f"sup{k}")
                    for k in range(NI // 2)]
            for m in range(NI):
                psg = psE.tile([128, 512], F32, tag="eg", bufs=2)
                for kk in range(DK // 2):
                    nc.tensor.matmul(psg, lhsT=wgu8[:, m, 0, kk, :, :],
                                     rhs=h2p[kk], start=(kk == 0),
                                     stop=(kk == DK // 2 - 1), perf_mode=DR)
                psu = psE.tile([128, 512], F32, tag="eu", bufs=2)
                for kk in range(DK // 2):
                    nc.tensor.matmul(psu, lhsT=wgu8[:, m, 1, kk, :, :],
                                     rhs=h2p[kk], start=(kk == 0),
                                     stop=(kk == DK // 2 - 1), perf_mode=DR)
                sg_t = pE.tile([128, 512], BF16, tag="sg_t", bufs=2)
                nc.scalar.activation(out=sg_t, in_=psg, func=Silu,
                                     scale=1.0 / SW, bias=inb_t[:, m:m + 1])
                tu = pE.tile([128, 512], F32, tag="tu", bufs=2)
                nc.vector.tensor_scalar(out=tu, in0=psu, scalar1=SU / SW,
                                        scalar2=inb_t[:, NI + m:NI + m + 1],
                                        op0=AluMult, op1=AluAdd)
                nc.vector.tensor_mul(su_p[m // 2][:, m % 2, :], tu, sg_t)
            # ---- w_out + final residual, straight to output ----
            for dm in range(DK):
                dmc = slice(dm * 128, (dm + 1) * 128)
                py2 = psE.tile([128, 512], F32, tag="ey2", bufs=2)
                for k in range(NI // 2):
                    nc.tensor.matmul(py2, lhsT=wout8[:, k, :, dmc],
                                     rhs=su_p[k], start=(k == 0),
                                     stop=(k == NI // 2 - 1), perf_mode=DR)
                yout = ld.tile([128, 512], F32, tag="yout", bufs=2)
                nc.vector.scalar_tensor_tensor(
                    out=yout, in0=py2, scalar=1.0 / (SW * SU),
                    in1=y1own[dm], op0=AluMult, op1=AluAdd)
                nc.sync.dma_start(out=yT_out[dm * 128:(dm + 1) * 128, :], in_=yout)
            psE_cm.__exit__(None, None, None)
            pE_cm.__exit__(None, None, None)
            pW_cm.__exit__(None, None, None)

    nc.compile()
    return nc


def _to_f8(a):
    return np.clip(a, -440.0, 440.0).astype(NP_F8)


def _prep_inputs(inputs):
    """Host-side shard prep: returns (lam, in_maps)."""
    f = {k: np.asarray(v, dtype=np.float32) for k, v in inputs.items()}
    lam = float(np.exp(np.sum(f["lq1"] * f["lk1"]))
                - np.exp(np.sum(f["lq2"] * f["lk2"])) + LAMBDA_INIT)
    x = f["x"].reshape(NS, D)
    x_bf = x.astype(NP_BF16)
    xT = np.ascontiguousarray(x.T)                       # [D, NS]
    # causal masks [pt, rel, cs]: allowed iff pt <= cs - 128*rel
    pt = np.arange(128)[:, None, None]
    rl = np.arange(4)[None, :, None]
    cs = np.arange(512)[None, None, :]
    masks = (pt <= cs - 128 * rl).astype(NP_BF16)
    ident = np.eye(128, dtype=NP_BF16)
    subln_base = (f["subln_w"] * (1.0 - LAMBDA_INIT) * SO).astype(np.float32)
    s8 = float(HD) ** -0.5
    l1w = f["ln1_w"][:, None]
    wq_e = l1w * f["wq"] * s8
    wk_e = l1w * f["wk"]
    wv_e = l1w * f["wv"]
    qb_full = f["ln1_b"] @ f["wq"] * s8                  # [D]
    kb_full = f["ln1_b"] @ f["wk"]
    vb_full = f["ln1_b"] @ f["wv"]
    w_in_e = f["ln2_w"][:, None] * f["w_in"]             # [D, 2*FFN] f32
    inb = (f["ln2_b"] @ f["w_in"]).astype(np.float32)    # [2*FFN]
    inb_sc = inb.copy()
    inb_sc[FFN:] *= SU                                   # up-bias pre-scaled
    # fp8 DoubleRow weight layouts (pre-scaled by SW)
    # wo8[p, h2, i, m] = wo[(2*h2+i)*128 + p, m] * SW
    wo8 = _to_f8((f["wo"] * SW).reshape(DK // 2, 2, 128, D)
                 .transpose(2, 0, 1, 3))
    # wgu8[p, m, g, kk0, i, c] = w_in_e[(2*kk0+i)*128+p, g*FFN + m*128+c] * SW
    wgu = (w_in_e * SW).reshape(DK // 2, 2, 128, 2, NI, 128)
    wgu8 = _to_f8(np.ascontiguousarray(wgu.transpose(2, 4, 3, 0, 1, 5)))
    # wout8[p, kk0, i, m] = w_out[(2*kk0+i)*128+p, m] * SW
    wout8 = _to_f8((f["w_out"] * SW).reshape(NI // 2, 2, 128, D)
                   .transpose(2, 0, 1, 3))
    in_maps = []
    for c in range(N_CORES):
        hc = slice(128 * c, 128 * (c + 1))
        # core c owns tokens [64c, 64c+64) of each (batch, sl) sigma block
        xo = np.concatenate(
            [xT[:, b * S + 512 * sl + 64 * c: b * S + 512 * sl + 64 * c + 64]
             for b in range(B) for sl in range(4)], axis=1)
        in_maps.append({
            "x_nat": x_bf,
            "xT_own": np.ascontiguousarray(xo),
            "wq_s": wq_e[:, hc].astype(NP_BF16),
            "wk_s": wk_e[:, hc].astype(NP_BF16),
            "wv_s": wv_e[:, hc].astype(NP_BF16),
            "wo8": wo8,
            "wgu8": wgu8,
            "wout8": wout8,
            "qb": np.ascontiguousarray(qb_full[hc]),
            "kb": np.ascontiguousarray(kb_full[hc]),
            "vb4": np.tile(vb_full[hc], 4).astype(NP_BF16),
            "inb": inb_sc,
            "subln_eff": subln_base,
            "masks": masks, "ident": ident,
        })
    return lam, in_maps


_CACHE = {}


def _run(inputs, trace=False, trace_kwargs=None):
    lam, in_maps = _prep_inputs(inputs)
    key = round(lam, 10)
    if key not in _CACHE:
        _CACHE[key] = build_program(lam)
    nc = _CACHE[key]
    res = bass_utils.run_bass_kernel_spmd(
        nc, in_maps, core_ids=list(range(N_CORES)),
        trace=trace, **(trace_kwargs or {}))
    y = np.empty((NS, D), dtype=np.float32)
    for c in range(N_CORES):
        yT = res.results[c]["yT"]                        # [D, 512]
        for b in range(B):
            for sl in range(4):
                fb = b * S + 512 * sl + 64 * c
                cb = (4 * b + sl) * 64
                y[fb:fb + 64, :] = yT[:, cb:cb + 64].T
    return y.reshape(B, S, D), res


def kernel(**inputs) -> np.ndarray:
    y, _ = _run(inputs)
    return y


# revision 35
# speedup vs baseline: 1.0919x; 1.0919x over previous
"""DiffTransformerLayer on 8 trn2 NeuronCores.

Tensor-parallel attention: core c owns diff-head c (softmax heads 2c, 2c+1).
The rank-128 subln outputs o_fin are exchanged with one fp8 AllToAll per
batch (the b=0 exchange hides under b=1's attention); each core then owns a
contiguous 2x256-token slice (sl=c//2, half=c%2 of each batch) and applies
wo / the FFN locally.

Everything on-chip lives in transposed ("T") layout [feature, token]: scores
are computed transposed so softmax / LN / RMS reductions over features run as
ones-vector matmuls on the TensorEngine.  LN1/LN2 scale+bias are folded into
the projection weights on the host (w' = diag(ln_w) @ w, bias = ln_b @ w).

Perf structure vs the bf16 baseline:
- wo / w_in / w_out matmuls run in fp8 (e4m3) DoubleRow perf mode: 2x128
  contraction rows per pass at 0.5 cycles/row.  Weights are pre-scaled by 64
  (e4m3 normal range), o_fin by 8 (folded into subln), silu(g)*u by 8
  (folded into the up-bias); the scales divide back out in the epilogues.
- softmax denominators are accumulated on the (otherwise idle) GpSimd and
  Vector engines instead of per-tau ones-matmuls on the PE.
- exp() for both softmax heads of a tau runs as ONE scalar activation over a
  2-bank PSUM tile.
- FFN + wo weights are fully prefetched into SBUF during attention.
"""

import sys

if "/opt/trn_rl_repo" not in sys.path:
    sys.path.insert(0, "/opt/trn_rl_repo")

import numpy as np

import concourse.bacc as bacc
import concourse.bass as bass
import concourse.tile as tile
from concourse import mybir
from concourse import bass_utils

F32 = mybir.dt.float32
F32R = mybir.dt.float32r
BF16 = mybir.dt.bfloat16
F8 = mybir.dt.float8e4
NP_BF16 = mybir.dt.np(BF16)
NP_F8 = mybir.dt.np(F8)

B, S, D = 2, 2048, 1024
H = 8
HD = 64
DEPTH = 12
LAMBDA_INIT = float(0.8 - 0.6 * np.exp(-0.3 * (DEPTH - 1)))
FFN = 2 * D
N_CORES = 8
NS = B * S                  # 4096 flattened tokens
NT = NS // 128              # 32 token tiles
DK = D // 128               # 8 feature tiles
NSIG = NS // 512            # 8 sigma blocks
NI = FFN // 128             # 16 inner-dim tiles
EPS = 1e-5
SW = 64.0                   # fp8 weight pre-scale
SO = 8.0                    # fp8 o_fin pre-scale (folded into subln)
SU = 8.0                    # fp8 silu(g)*u pre-scale (folded into up-bias)
Exp = mybir.ActivationFunctionType.Exp
Sqrt = mybir.ActivationFunctionType.Sqrt
Silu = mybir.ActivationFunctionType.Silu
Ident = mybir.ActivationFunctionType.Identity
AluAdd = mybir.AluOpType.add
AluSub = mybir.AluOpType.subtract
AluMult = mybir.AluOpType.mult
DR = mybir.MatmulPerfMode.DoubleRow
RG = [list(range(N_CORES))]


def build_program(lam: float):
    nc = bacc.Bacc("TRN2", target_bir_lowering=False, debug=False,
                   enable_asserts=False, num_devices=N_CORES)

    # ---- external I/O (identical shapes on every core) ----
    x_nat = nc.dram_tensor("x_nat", [NS, D], BF16, kind="ExternalInput").ap()
    xT_own = nc.dram_tensor("xT_own", [D, 512], F32, kind="ExternalInput").ap()
    wq_s = nc.dram_tensor("wq_s", [D, 128], BF16, kind="ExternalInput").ap()
    wk_s = nc.dram_tensor("wk_s", [D, 128], BF16, kind="ExternalInput").ap()
    wv_s = nc.dram_tensor("wv_s", [D, 128], BF16, kind="ExternalInput").ap()
    # fp8 paired weights (DoubleRow layouts, already x SW)
    wo8_in = nc.dram_tensor("wo8", [128, DK // 2, 2, D], F8,
                            kind="ExternalInput").ap()
    wgu8_in = nc.dram_tensor("wgu8", [128, NI, 2, DK // 2, 2, 128], F8,
                             kind="ExternalInput").ap()
    wout8_in = nc.dram_tensor("wout8", [128, NI // 2, 2, D], F8,
                              kind="ExternalInput").ap()
    qb_in = nc.dram_tensor("qb", [128], F32, kind="ExternalInput").ap()
    kb_in = nc.dram_tensor("kb", [128], F32, kind="ExternalInput").ap()
    vb4_in = nc.dram_tensor("vb4", [512], BF16, kind="ExternalInput").ap()
    inb_in = nc.dram_tensor("inb", [2 * FFN], F32, kind="ExternalInput").ap()
    subln_eff = nc.dram_tensor("subln_eff", [128], F32, kind="ExternalInput").ap()
    masks_in = nc.dram_tensor("masks", [128, 4, 512], BF16, kind="ExternalInput").ap()
    ident_in = nc.dram_tensor("ident", [128, 128], BF16, kind="ExternalInput").ap()
    yT_out = nc.dram_tensor("yT", [D, 512], F32, kind="ExternalOutput").ap()
    import os
    DBG = os.environ.get("KDBG", "0") == "1"
    if DBG:
        dbg_out = nc.dram_tensor("dbg", [128, 12, 512], BF16,
                                 kind="ExternalOutput").ap()

    with tile.TileContext(nc) as tc:
        with (
            tc.tile_pool(name="persist", bufs=1) as persist,
            tc.tile_pool(name="ld", bufs=1) as ld,
            tc.tile_pool(name="stats", bufs=2) as stats,
            tc.tile_pool(name="dram", bufs=1, space="DRAM") as dram,
        ):
            # ---- constants / small inputs ----
            ones_c = persist.tile([128, 1], BF16, tag="ones_c")
            nc.vector.memset(ones_c, 1.0)
            rowinit = persist.tile([1, 128], F32, tag="rowinit")
            ones_rf = persist.tile([1, 128], F32R, tag="ones_rf")
            nc.vector.memset(rowinit, 1.0)
            with nc.allow_low_precision(reason="f32r constant rows"):
                nc.vector.tensor_copy(ones_rf, rowinit)
            ones_rb = persist.tile([1, 128], BF16, tag="ones_rb")
            nc.vector.memset(ones_rb, 1.0)
            rowinit2 = persist.tile([1, 128], F32, tag="rowinit2")
            lam_r = persist.tile([1, 128], F32R, tag="lam_r")
            nc.vector.memset(rowinit2, float(lam))
            with nc.allow_low_precision(reason="f32r constant rows"):
                nc.vector.tensor_copy(lam_r, rowinit2)
            eps128 = persist.tile([128, 1], F32, tag="eps128")
            nc.vector.memset(eps128, EPS)
            eps1 = persist.tile([1, 1], F32, tag="eps1")
            nc.vector.memset(eps1, EPS)
            subln_t = persist.tile([128, 1], F32, tag="subln")
            nc.sync.dma_start(out=subln_t,
                              in_=subln_eff.rearrange("(p one) -> p one", one=1))
            qb_t = persist.tile([128, 1], F32, tag="qb_t")
            nc.sync.dma_start(out=qb_t, in_=qb_in.rearrange("(p one) -> p one", one=1))
            kb_t = persist.tile([128, 1], F32, tag="kb_t")
            nc.sync.dma_start(out=kb_t, in_=kb_in.rearrange("(p one) -> p one", one=1))
            vb4_r = persist.tile([1, 512], BF16, tag="vb4_r")
            nc.sync.dma_start(out=vb4_r, in_=vb4_in.rearrange("(one f) -> one f", one=1))
            inb_t = persist.tile([128, 2 * NI], F32, tag="inb_t")
            nc.sync.dma_start(out=inb_t, in_=inb_in.rearrange("(k p) -> p k", p=128))

            # ---- fp8 weights + A2A landing (D/E lifetime; DMAs issued post-B) ----
            pW_cm = tc.tile_pool(name="pW", bufs=1)
            pW = pW_cm.__enter__()
            wo8 = pW.tile([128, DK // 2, 2, D], F8, tag="wo8")
            wgu8 = pW.tile([128, NI, 2, DK // 2, 2, 128], F8, tag="wgu8")
            wout8 = pW.tile([128, NI // 2, 2, D], F8, tag="wout8")
            af = [pW.tile([128, DK, 4, 64], F8, tag=f"af{b}", name=f"af{b}")
                  for b in range(B)]

            # ---- stage A/B/C lifetime pool ----
            pqkv_cm = tc.tile_pool(name="pqkv", bufs=1)
            pqkv = pqkv_cm.__enter__()
            qT = [pqkv.tile([128, 512], BF16, tag=f"qT{s}", name=f"qT{s}")
                  for s in range(NSIG)]
            kT = [pqkv.tile([128, 512], BF16, tag=f"kT{s}", name=f"kT{s}")
                  for s in range(NSIG)]
            v_t = [pqkv.tile([128, 512], BF16, tag=f"v{s}", name=f"v{s}")
                   for s in range(NSIG)]
            ident = pqkv.tile([128, 128], BF16, tag="ident")
            nc.sync.dma_start(out=ident, in_=ident_in)
            masks = pqkv.tile([128, 4, 512], BF16, tag="masks")
            nc.sync.dma_start(out=masks, in_=masks_in)
            wq_sb = pqkv.tile([128, D], BF16, tag="wq_sb")
            wk_sb = pqkv.tile([128, D], BF16, tag="wk_sb")
            wv_sb = pqkv.tile([128, D], BF16, tag="wv_sb")
            # qkv weights land before the bulk x stream
            for sb_t, wsrc in ((wq_sb, wq_s), (wk_sb, wk_s), (wv_sb, wv_s)):
                nc.sync.dma_start(
                    out=sb_t.rearrange("p (k m) -> p k m", m=128),
                    in_=wsrc.rearrange("(k p) m -> p k m", p=128))

            # AllToAll bounce buffers, one per (b, sl) sigma block: chunk u of
            # the input is o_fin[:, 64u:64u+64]; after the exchange, out[h] is
            # head h's o_fin for OUR 64-token unit of that sigma block.
            a2a_in = [dram.tile([N_CORES, 128, 2, 64], F8, tag=f"a2ai{g}",
                                name=f"a2ai{g}") for g in range(4)]
            a2a_out = [dram.tile([N_CORES, 128, 2, 64], F8, tag=f"a2ao{g}",
                                 name=f"a2ao{g}") for g in range(4)]

            # shared PSUM pool for stages A+B+C (8 banks exactly)
            psC_cm = tc.tile_pool(name="psC", bufs=1, space="PSUM")
            psC = psC_cm.__enter__()

            # v bias broadcast [128, 512] (4 repeats of the 128-wide bias row)
            pbv = psC.tile([128, 512], F32, tag="o1")
            nc.tensor.matmul(pbv, lhsT=ones_rb, rhs=vb4_r, start=True, stop=True)
            bv_bc = pqkv.tile([128, 512], F32, tag="bv_bc")
            nc.vector.tensor_copy(bv_bc, pbv)

            # ================= Stage A: LN1 + transpose =================
            # hT is a rotating per-sigma pipeline: stage B only reads the
            # tiles of its own sigma block, so 3 bufs per group suffice.
            phT_cm = tc.tile_pool(name="phT", bufs=1)
            phT = phT_cm.__enter__()
            hTs = []
            for s8 in range(NSIG):
                hTg = [phT.tile([128, 4, 512], BF16, tag=f"hT{g}", bufs=3,
                                name=f"hT{g}_{s8}") for g in range(2)]
                hTs.append(hTg)
                mvg = stats.tile([128, 4, 2], F32, tag="mvg")
                x4 = []
                for j4 in range(4):
                    st = s8 * 4 + j4
                    x_t = ld.tile([128, D], BF16, tag="x_t", bufs=6)
                    nc.sync.dma_start(out=x_t, in_=x_nat[st * 128:(st + 1) * 128, :])
                    st_t = stats.tile([128, 2, 6], F32, tag="bst")
                    xg = x_t.rearrange("p (g d) -> p g d", g=2)
                    for g in range(2):
                        nc.vector.bn_stats(out=st_t[:, g, :], in_=xg[:, g, :])
                    nc.vector.bn_aggr(out=mvg[:, j4, :], in_=st_t)
                    x4.append(x_t)
                rstd4 = stats.tile([128, 4], F32, tag="rstd4")
                nc.scalar.activation(out=rstd4, in_=mvg[:, :, 1], func=Sqrt,
                                     bias=eps128, scale=1.0)
                nc.vector.reciprocal(out=rstd4, in_=rstd4)
                for j4 in range(4):
                    st = s8 * 4 + j4
                    h_t = ld.tile([128, D], BF16, tag="h_t", bufs=4)
                    nc.vector.tensor_scalar(out=h_t, in0=x4[j4],
                                            scalar1=mvg[:, j4, 0:1],
                                            scalar2=rstd4[:, j4:j4 + 1],
                                            op0=AluSub, op1=AluMult)
                    jcol = slice(j4 * 128, (j4 + 1) * 128)
                    for g4 in range(2):
                        tpw = psC.tile([128, 1024], BF16, tag="tp", bufs=2, name="tp")
                        tp = tpw[:, 0:512]
                        for j in range(4):
                            dk = g4 * 4 + j
                            nc.tensor.transpose(tp[:, j * 128:(j + 1) * 128],
                                                h_t[:, dk * 128:(dk + 1) * 128], ident)
                        dst = hTs[s8][g4][:, :, jcol]
                        srcv = tp.rearrange("p (j f) -> p j f", f=128)
                        if (st + g4) % 2 == 0:
                            nc.vector.tensor_copy(dst, srcv)
                        else:
                            nc.scalar.copy(dst, srcv)

            # ================= Stage B: q,k,v projections =================
            for sg in range(NSIG):
                psq = psC.tile([128, 512], F32, tag="s12", bufs=2)
                for kk in range(DK):
                    nc.tensor.matmul(psq, lhsT=wq_sb[:, kk * 128:(kk + 1) * 128],
                                     rhs=hTs[sg][kk // 4][:, kk % 4, :],
                                     start=(kk == 0), stop=(kk == DK - 1))
                nc.scalar.activation(out=qT[sg], in_=psq, func=Ident,
                                     scale=1.0, bias=qb_t)
                psk = psC.tile([128, 512], F32, tag="s12", bufs=2)
                for kk in range(DK):
                    nc.tensor.matmul(psk, lhsT=wk_sb[:, kk * 128:(kk + 1) * 128],
                                     rhs=hTs[sg][kk // 4][:, kk % 4, :],
                                     start=(kk == 0), stop=(kk == DK - 1))
                nc.scalar.activation(out=kT[sg], in_=psk, func=Ident,
                                     scale=1.0, bias=kb_t)
                psv = psC.tile([128, 512], F32, tag="s12", bufs=2)
                for j4 in range(4):
                    for kk in range(DK):
                        nc.tensor.matmul(psv[:, j4 * 128:(j4 + 1) * 128],
                                         lhsT=hTs[sg][kk // 4][:, kk % 4, j4 * 128:(j4 + 1) * 128],
                                         rhs=wv_sb[:, kk * 128:(kk + 1) * 128],
                                         start=(kk == 0), stop=(kk == DK - 1))
                nc.vector.tensor_add(v_t[sg], psv, bv_bc)
            phT_cm.__exit__(None, None, None)

            # ---- weight prefetch (overlaps attention) ----
            nc.sync.dma_start(out=wo8, in_=wo8_in)
            nc.sync.dma_start(out=wgu8, in_=wgu8_in)
            nc.sync.dma_start(out=wout8, in_=wout8_in)

            # ================= Stage C: differential attention =================
            pwc_cm = tc.tile_pool(name="pwc", bufs=1)
            pwc = pwc_cm.__enter__()
            for b in range(B):
                for sl in (0, 1, 2, 3):
                    sg = 4 * b + sl
                    ntau = 4 * (sl + 1)
                    o1 = psC.tile([128, 512], F32, tag="o1")
                    o2 = psC.tile([128, 512], F32, tag="o2")
                    esum1 = pwc.tile([128, 512], BF16, tag="es1", bufs=2)
                    esum2 = pwc.tile([128, 512], BF16, tag="es2", bufs=2)
                    for tau in range(ntau):
                        tg = 16 * b + tau
                        ts8, tj = tg // 4, tg % 4
                        tcol = slice(tj * 128, (tj + 1) * 128)
                        rel = tau - 4 * sl
                        off = max(rel, 0) * 128          # causal column offset
                        ecol = slice(off, 512)
                        st_fl = (tau == 0)
                        sp_fl = (tau == ntau - 1)
                        s12 = psC.tile([128, 2, 512], F32, tag="s12", bufs=2)
                        nc.tensor.matmul(s12[:, 0, ecol], lhsT=kT[ts8][0:64, tcol],
                                         rhs=qT[sg][0:64, ecol], start=True, stop=True)
                        nc.tensor.matmul(s12[:, 1, ecol], lhsT=kT[ts8][64:128, tcol],
                                         rhs=qT[sg][64:128, ecol], start=True, stop=True)
                        e12 = pwc.tile([128, 2, 512], BF16, tag="e12", bufs=6)
                        nc.scalar.activation(out=e12[:, :, ecol], in_=s12[:, :, ecol],
                                             func=Exp)
                        e1 = e12[:, 0, :]
                        e2 = e12[:, 1, :]
                        if rel >= 0:
                            # only the 128-wide diagonal strip needs masking
                            strip = slice(off, off + 128)
                            tri = masks[:, 0, 0:128]
                            nc.gpsimd.tensor_mul(e1[:, strip], e1[:, strip], tri)
                            nc.vector.tensor_mul(e2[:, strip], e2[:, strip], tri)
                        if st_fl:
                            nc.gpsimd.tensor_copy(esum1, e1)
                            nc.vector.tensor_copy(esum2, e2)
                        else:
                            nc.gpsimd.tensor_add(esum1[:, ecol], e1[:, ecol],
                                                 esum1[:, ecol])
                            nc.vector.tensor_add(esum2[:, ecol], e2[:, ecol],
                                                 esum2[:, ecol])
                        nc.tensor.matmul(o1[:, ecol], lhsT=v_t[ts8][:, tcol],
                                         rhs=e1[:, ecol], start=st_fl, stop=sp_fl)
                        nc.tensor.matmul(o2[:, ecol], lhsT=v_t[ts8][:, tcol],
                                         rhs=e2[:, ecol], start=st_fl, stop=sp_fl)
                    # ---- differential combine + subln ----
                    z1 = psC.tile([1, 512], F32, tag="tp", bufs=2)
                    nc.tensor.matmul(z1, lhsT=ones_c, rhs=esum1, start=True, stop=True)
                    z2 = psC.tile([1, 512], F32, tag="tp", bufs=2)
                    nc.tensor.matmul(z2, lhsT=ones_c, rhs=esum2, start=True, stop=True)
                    zrec = stats.tile([1, 512], F32, tag="zrec")
                    nc.vector.reciprocal_approx_fast(out=zrec, in_=z2)
                    zr = stats.tile([1, 512], F32R, tag="rowf1")
                    with nc.allow_low_precision(reason="softmax ratio to f32r row"):
                        nc.vector.tensor_mul(zr, z1, zrec)
                    w_bc = psC.tile([128, 512], F32, tag="tp", bufs=2)
                    nc.tensor.matmul(w_bc, lhsT=lam_r, rhs=zr, start=True, stop=True)
                    w_sb = pwc.tile([128, 512], F32, tag="w_sb")
                    nc.vector.tensor_copy(w_sb, w_bc)
                    t_sb = pwc.tile([128, 512], F32, tag="t_sb")
                    nc.vector.tensor_mul(t_sb, o2, w_sb)
                    oc = pwc.tile([128, 512], F32, tag="oc")
                    nc.vector.tensor_sub(oc, o1, t_sb)
                    sq = pwc.tile([128, 512], BF16, tag="sq")
                    nc.gpsimd.tensor_mul(sq, oc, oc)
                    ss = psC.tile([1, 512], F32, tag="tp", bufs=2)
                    nc.tensor.matmul(ss, lhsT=ones_c, rhs=sq, start=True, stop=True)
                    rt = stats.tile([1, 512], F32, tag="rt")
                    nc.scalar.activation(out=rt, in_=ss, func=Sqrt,
                                         scale=1.0 / 128.0, bias=eps1)
                    rrf = stats.tile([1, 512], F32, tag="rowf3")
                    nc.vector.reciprocal_approx_fast(out=rrf, in_=rt)
                    rr = stats.tile([1, 512], F32R, tag="rowf2")
                    with nc.allow_low_precision(reason="rms recip to f32r row"):
                        nc.vector.tensor_copy(rr, rrf)
                    r_bc = psC.tile([128, 512], F32, tag="tp", bufs=2)
                    nc.tensor.matmul(r_bc, lhsT=ones_rf, rhs=rr, start=True, stop=True)
                    t2 = pwc.tile([128, 512], F32, tag="t2")
                    nc.vector.tensor_mul(t2, oc, r_bc)
                    o_fin = pwc.tile([128, 512], F8, tag="o_fin", bufs=4)
                    nc.vector.tensor_scalar_mul(o_fin, t2, subln_t)
                    grp = 2 * b + sl // 2
                    nc.sync.dma_start(
                        out=a2a_in[grp][:, :, sl % 2, :].rearrange("u p f -> p u f"),
                        in_=o_fin.rearrange("p (u f) -> p u f", f=64))
                    if sl % 2 == 1:
                        # AllToAll per sigma pair: all but the last overlap
                        # the remaining attention compute
                        nc.gpsimd.collective_compute(
                            "AllToAll", mybir.AluOpType.bypass, replica_groups=RG,
                            ins=[a2a_in[grp].opt()], outs=[a2a_out[grp].opt()])
                if DBG and b == 0:
                    dv = dbg_out.rearrange("p s f -> p (s f)")
                    nc.sync.dma_start(
                        out=dv[:, 0:2048].rearrange("p (u f) -> u p f", f=256),
                        in_=a2a_in[0])
                    nc.sync.dma_start(
                        out=dv[:, 2048:4096].rearrange("p (u f) -> u p f", f=256),
                        in_=a2a_out[0])
                    nc.sync.dma_start(
                        out=dv[:, 4096:6144].rearrange("p (u f) -> p u f", f=256),
                        in_=af[0])
            pwc_cm.__exit__(None, None, None)
            psC_cm.__exit__(None, None, None)
            pqkv_cm.__exit__(None, None, None)

            # ================= Stage D: local wo + residuals =================
            pE_cm = tc.tile_pool(name="pE", bufs=1)
            pE = pE_cm.__enter__()
            psE_cm = tc.tile_pool(name="psE", bufs=1, space="PSUM")
            psE = psE_cm.__enter__()
            for b in range(B):
                for sl in range(4):
                    nc.sync.dma_start(
                        out=af[b][:, :, sl, :],
                        in_=a2a_out[2 * b + sl // 2][:, :, sl % 2, :]
                        .rearrange("h p f -> p h f"))
            y1own = [pE.tile([128, 512], F32, tag=f"y1own{dk}", name=f"y1own{dk}")
                     for dk in range(DK)]
            y1bf = [pE.tile([128, 512], BF16, tag=f"y1bf{dk}", name=f"y1bf{dk}")
                    for dk in range(DK)]
            for dm in range(DK):
                xo_t = ld.tile([128, 512], F32, tag="xo_t", bufs=2)
                nc.sync.dma_start(out=xo_t, in_=xT_own[dm * 128:(dm + 1) * 128, :])
                dmc = slice(dm * 128, (dm + 1) * 128)
                for b in range(B):
                    hcol = slice(b * 256, (b + 1) * 256)
                    pwo = psE.tile([128, 256], F32, tag="ey2", bufs=2)
                    afp = af[b].rearrange("p (h2 i) s f -> p h2 i (s f)", i=2)
                    for h2 in range(DK // 2):
                        nc.tensor.matmul(pwo, lhsT=wo8[:, h2, :, dmc],
                                         rhs=afp[:, h2], start=(h2 == 0),
                                         stop=(h2 == DK // 2 - 1), perf_mode=DR)
                    nc.vector.scalar_tensor_tensor(
                        out=y1own[dm][:, hcol], in0=pwo, scalar=1.0 / (SW * SO),
                        in1=xo_t[:, hcol], op0=AluMult, op1=AluAdd)
                    with nc.allow_low_precision(reason="ffn input is bf16"):
                        nc.vector.scalar_tensor_tensor(
                            out=y1bf[dm][:, hcol], in0=pwo, scalar=1.0 / (SW * SO),
                            in1=xo_t[:, hcol], op0=AluMult, op1=AluAdd)

            # ================= Stage E: LN2 + FFN (local) =================
            ssum = psE.tile([1, 512], F32, tag="es")
            ssq = psE.tile([1, 512], F32, tag="esq")
            for dk in range(DK):
                nc.tensor.matmul(ssum, lhsT=ones_c, rhs=y1bf[dk],
                                 start=(dk == 0), stop=(dk == DK - 1))
                sqt = ld.tile([128, 512], BF16, tag="sqt", bufs=2)
                nc.vector.tensor_mul(sqt, y1bf[dk], y1bf[dk])
                nc.tensor.matmul(ssq, lhsT=ones_c, rhs=sqt,
                                 start=(dk == 0), stop=(dk == DK - 1))
            m_row = stats.tile([1, 512], F32, tag="rowf1")
            nc.vector.tensor_scalar_mul(m_row, ssum, 1.0 / float(D))
            mm_row = stats.tile([1, 512], F32, tag="rowf2")
            nc.vector.tensor_mul(mm_row, m_row, m_row)
            v_row = stats.tile([1, 512], F32, tag="rowf3")
            nc.vector.tensor_scalar_mul(v_row, ssq, 1.0 / float(D))
            nc.vector.tensor_sub(v_row, v_row, mm_row)
            nc.scalar.activation(out=v_row, in_=v_row, func=Sqrt,
                                 scale=1.0, bias=eps1)
            r_row = stats.tile([1, 512], F32R, tag="rowf4")
            mr_row = stats.tile([1, 512], F32R, tag="rowf5")
            with nc.allow_low_precision(reason="ln2 rows to f32r"):
                nc.vector.reciprocal(out=r_row, in_=v_row)
                nc.vector.tensor_mul(mr_row, m_row, r_row)
            pbc = psE.tile([128, 512], F32, tag="es")
            nc.tensor.matmul(pbc, lhsT=ones_rf, rhs=r_row, start=True, stop=True)
            rbc2 = pE.tile([128, 512], BF16, tag="rbc2")
            nc.vector.tensor_copy(rbc2, pbc)
            pbc2 = psE.tile([128, 512], F32, tag="esq")
            nc.tensor.matmul(pbc2, lhsT=ones_rf, rhs=mr_row, start=True, stop=True)
            mrbc = pE.tile([128, 512], BF16, tag="mrbc")
            nc.vector.tensor_copy(mrbc, pbc2)
            # h2 in fp8, paired along the contraction dim for DoubleRow
            h2p = [pE.tile([128, 2, 512], F8, tag=f"h2p{k}", name=f"h2p{k}")
                   for k in range(DK // 2)]
            for dk in range(DK):
                a = ld.tile([128, 512], BF16, tag="h2t", bufs=2)
                nc.vector.tensor_mul(a, y1bf[dk], rbc2)
                nc.gpsimd.tensor_sub(h2p[dk // 2][:, dk % 2, :], a, mrbc)
            su_p = [pE.tile([128, 2, 512], F8, tag=f"sup{k}", name=f"sup{k}")
                    for k in range(NI // 2)]
            for m in range(NI):
                psg = psE.tile([128, 512], F32, tag="eg", bufs=2)
                for kk in range(DK // 2):
                    nc.tensor.matmul(psg, lhsT=wgu8[:, m, 0, kk, :, :],
                                     rhs=h2p[kk], start=(kk == 0),
                                     stop=(kk == DK // 2 - 1), perf_mode=DR)
                psu = psE.tile([128, 512], F32, tag="eu", bufs=2)
                for kk in range(DK // 2):
                    nc.tensor.matmul(psu, lhsT=wgu8[:, m, 1, kk, :, :],
                                     rhs=h2p[kk], start=(kk == 0),
                                     stop=(kk == DK // 2 - 1), perf_mode=DR)
                sg_t = pE.tile([128, 512], BF16, tag="sg_t", bufs=2)
                nc.scalar.activation(out=sg_t, in_=psg, func=Silu,
                                     scale=1.0 / SW, bias=inb_t[:, m:m + 1])
                tu = pE.tile([128, 512], F32, tag="tu", bufs=2)
                nc.vector.tensor_scalar(out=tu, in0=psu, scalar1=SU / SW,
                                        scalar2=inb_t[:, NI + m:NI + m + 1],
                                        op0=AluMult, op1=AluAdd)
                nc.vector.tensor_mul(su_p[m // 2][:, m % 2, :], tu, sg_t)
            # ---- w_out + final residual, straight to output ----
            for dm in range(DK):
                dmc = slice(dm * 128, (dm + 1) * 128)
                py2 = psE.tile([128, 512], F32, tag="ey2", bufs=2)
                for k in range(NI // 2):
                    nc.tensor.matmul(py2, lhsT=wout8[:, k, :, dmc],
                                     rhs=su_p[k], start=(k == 0),
                                     stop=(k == NI // 2 - 1), perf_mode=DR)
                yout = ld.tile([128, 512], F32, tag="yout", bufs=2)
                nc.vector.scalar_tensor_tensor(
                    out=yout, in0=py2, scalar=1.0 / (SW * SU),
                    in1=y1own[dm], op0=AluMult, op1=AluAdd)
                nc.sync.dma_start(out=yT_out[dm * 128:(dm + 1) * 128, :], in_=yout)
            psE_cm.__exit__(None, None, None)
            pE_cm.__exit__(None, None, None)
            pW_cm.__exit__(None, None, None)

    nc.compile()
    return nc


def _to_f8(a):
    return np.clip(a, -440.0, 440.0).astype(NP_F8)


def _prep_inputs(inputs):
    """Host-side shard prep: returns (lam, in_maps)."""
    f = {k: np.asarray(v, dtype=np.float32) for k, v in inputs.items()}
    lam = float(np.exp(np.sum(f["lq1"] * f["lk1"]))
                - np.exp(np.sum(f["lq2"] * f["lk2"])) + LAMBDA_INIT)
    x = f["x"].reshape(NS, D)
    x_bf = x.astype(NP_BF16)
    xT = np.ascontiguousarray(x.T)                       # [D, NS]
    # causal masks [pt, rel, cs]: allowed iff pt <= cs - 128*rel
    pt = np.arange(128)[:, None, None]
    rl = np.arange(4)[None, :, None]
    cs = np.arange(512)[None, None, :]
    masks = (pt <= cs - 128 * rl).astype(NP_BF16)
    ident = np.eye(128, dtype=NP_BF16)
    subln_base = (f["subln_w"] * (1.0 - LAMBDA_INIT) * SO).astype(np.float32)
    s8 = float(HD) ** -0.5
    l1w = f["ln1_w"][:, None]
    wq_e = l1w * f["wq"] * s8
    wk_e = l1w * f["wk"]
    wv_e = l1w * f["wv"]
    qb_full = f["ln1_b"] @ f["wq"] * s8                  # [D]
    kb_full = f["ln1_b"] @ f["wk"]
    vb_full = f["ln1_b"] @ f["wv"]
    w_in_e = f["ln2_w"][:, None] * f["w_in"]             # [D, 2*FFN] f32
    inb = (f["ln2_b"] @ f["w_in"]).astype(np.float32)    # [2*FFN]
    inb_sc = inb.copy()
    inb_sc[FFN:] *= SU                                   # up-bias pre-scaled
    # fp8 DoubleRow weight layouts (pre-scaled by SW)
    # wo8[p, h2, i, m] = wo[(2*h2+i)*128 + p, m] * SW
    wo8 = _to_f8((f["wo"] * SW).reshape(DK // 2, 2, 128, D)
                 .transpose(2, 0, 1, 3))
    # wgu8[p, m, g, kk0, i, c] = w_in_e[(2*kk0+i)*128+p, g*FFN + m*128+c] * SW
    wgu = (w_in_e * SW).reshape(DK // 2, 2, 128, 2, NI, 128)
    wgu8 = _to_f8(np.ascontiguousarray(wgu.transpose(2, 4, 3, 0, 1, 5)))
    # wout8[p, kk0, i, m] = w_out[(2*kk0+i)*128+p, m] * SW
    wout8 = _to_f8((f["w_out"] * SW).reshape(NI // 2, 2, 128, D)
                   .transpose(2, 0, 1, 3))
    in_maps = []
    for c in range(N_CORES):
        hc = slice(128 * c, 128 * (c + 1))
        # core c owns tokens [64c, 64c+64) of each (batch, sl) sigma block
        xo = np.concatenate(
            [xT[:, b * S + 512 * sl + 64 * c: b * S + 512 * sl + 64 * c + 64]
             for b in range(B) for sl in range(4)], axis=1)
        in_maps.append({
            "x_nat": x_bf,
            "xT_own": np.ascontiguousarray(xo),
            "wq_s": wq_e[:, hc].astype(NP_BF16),
            "wk_s": wk_e[:, hc].astype(NP_BF16),
            "wv_s": wv_e[:, hc].astype(NP_BF16),
            "wo8": wo8,
            "wgu8": wgu8,
            "wout8": wout8,
            "qb": np.ascontiguousarray(qb_full[hc]),
            "kb": np.ascontiguousarray(kb_full[hc]),
            "vb4": np.tile(vb_full[hc], 4).astype(NP_BF16),
            "inb": inb_sc,
            "subln_eff": subln_base,
            "masks": masks, "ident": ident,
        })
    return lam, in_maps


_CACHE = {}


def _run(inputs, trace=False, trace_kwargs=None):
    lam, in_maps = _prep_inputs(inputs)
    key = round(lam, 10)
    if key not in _CACHE:
        _CACHE[key] = build_program(lam)
    nc = _CACHE[key]
    res = bass_utils.run_bass_kernel_spmd(
        nc, in_maps, core_ids=list(range(N_CORES)),
        trace=trace, **(trace_kwargs or {}))
    y = np.empty((NS, D), dtype=np.float32)
    for c in range(N_CORES):
        yT = res.results[c]["yT"]                        # [D, 512]
        for b in range(B):
            for sl in range(4):
                fb = b * S + 512 * sl + 64 * c
                cb = (4 * b + sl) * 64
                y[fb:fb + 64, :] = yT[:, cb:cb + 64].T
    return y.reshape(B, S, D), res


def kernel(**inputs) -> np.ndarray:
    y, _ = _run(inputs)
    return y


# revision 37
# speedup vs baseline: 1.0991x; 1.0066x over previous
"""DiffTransformerLayer on 8 trn2 NeuronCores.

Tensor-parallel attention: core c owns diff-head c (softmax heads 2c, 2c+1).
The rank-128 subln outputs o_fin are exchanged with one fp8 AllToAll per
batch (the b=0 exchange hides under b=1's attention); each core then owns a
contiguous 2x256-token slice (sl=c//2, half=c%2 of each batch) and applies
wo / the FFN locally.

Everything on-chip lives in transposed ("T") layout [feature, token]: scores
are computed transposed so softmax / LN / RMS reductions over features run as
ones-vector matmuls on the TensorEngine.  LN1/LN2 scale+bias are folded into
the projection weights on the host (w' = diag(ln_w) @ w, bias = ln_b @ w).

Perf structure vs the bf16 baseline:
- wo / w_in / w_out matmuls run in fp8 (e4m3) DoubleRow perf mode: 2x128
  contraction rows per pass at 0.5 cycles/row.  Weights are pre-scaled by 64
  (e4m3 normal range), o_fin by 8 (folded into subln), silu(g)*u by 8
  (folded into the up-bias); the scales divide back out in the epilogues.
- softmax denominators are accumulated on the (otherwise idle) GpSimd and
  Vector engines instead of per-tau ones-matmuls on the PE.
- exp() for both softmax heads of a tau runs as ONE scalar activation over a
  2-bank PSUM tile.
- FFN + wo weights are fully prefetched into SBUF during attention.
"""

import sys

if "/opt/trn_rl_repo" not in sys.path:
    sys.path.insert(0, "/opt/trn_rl_repo")

import numpy as np

import concourse.bacc as bacc
import concourse.bass as bass
import concourse.tile as tile
from concourse import mybir
from concourse import bass_utils

F32 = mybir.dt.float32
F32R = mybir.dt.float32r
BF16 = mybir.dt.bfloat16
F8 = mybir.dt.float8e4
NP_BF16 = mybir.dt.np(BF16)
NP_F8 = mybir.dt.np(F8)

B, S, D = 2, 2048, 1024
H = 8
HD = 64
DEPTH = 12
LAMBDA_INIT = float(0.8 - 0.6 * np.exp(-0.3 * (DEPTH - 1)))
FFN = 2 * D
N_CORES = 8
NS = B * S                  # 4096 flattened tokens
NT = NS // 128              # 32 token tiles
DK = D // 128               # 8 feature tiles
NSIG = NS // 512            # 8 sigma blocks
NI = FFN // 128             # 16 inner-dim tiles
EPS = 1e-5
SW = 64.0                   # fp8 weight pre-scale
SO = 8.0                    # fp8 o_fin pre-scale (folded into subln)
SU = 8.0                    # fp8 silu(g)*u pre-scale (folded into up-bias)
Exp = mybir.ActivationFunctionType.Exp
Sqrt = mybir.ActivationFunctionType.Sqrt
Silu = mybir.ActivationFunctionType.Silu
Ident = mybir.ActivationFunctionType.Identity
AluAdd = mybir.AluOpType.add
AluSub = mybir.AluOpType.subtract
AluMult = mybir.AluOpType.mult
DR = mybir.MatmulPerfMode.DoubleRow
RG = [list(range(N_CORES))]


def build_program(lam: float):
    nc = bacc.Bacc("TRN2", target_bir_lowering=False, debug=False,
                   enable_asserts=False, num_devices=N_CORES)

    # ---- external I/O (identical shapes on every core) ----
    x_nat = nc.dram_tensor("x_nat", [NS, D], BF16, kind="ExternalInput").ap()
    xT_own = nc.dram_tensor("xT_own", [D, 512], F32, kind="ExternalInput").ap()
    wq_s = nc.dram_tensor("wq_s", [D, 128], BF16, kind="ExternalInput").ap()
    wk_s = nc.dram_tensor("wk_s", [D, 128], BF16, kind="ExternalInput").ap()
    wv_s = nc.dram_tensor("wv_s", [D, 128], BF16, kind="ExternalInput").ap()
    # fp8 paired weights (DoubleRow layouts, already x SW)
    wo8_in = nc.dram_tensor("wo8", [128, DK // 2, 2, D], F8,
                            kind="ExternalInput").ap()
    wgu8_in = nc.dram_tensor("wgu8", [128, NI, 2, DK // 2, 2, 128], F8,
                             kind="ExternalInput").ap()
    wout8_in = nc.dram_tensor("wout8", [128, NI // 2, 2, D], F8,
                              kind="ExternalInput").ap()
    qb_in = nc.dram_tensor("qb", [128], F32, kind="ExternalInput").ap()
    kb_in = nc.dram_tensor("kb", [128], F32, kind="ExternalInput").ap()
    vb4_in = nc.dram_tensor("vb4", [512], BF16, kind="ExternalInput").ap()
    inb_in = nc.dram_tensor("inb", [2 * FFN], F32, kind="ExternalInput").ap()
    subln_eff = nc.dram_tensor("subln_eff", [128], F32, kind="ExternalInput").ap()
    masks_in = nc.dram_tensor("masks", [128, 4, 512], BF16, kind="ExternalInput").ap()
    ident_in = nc.dram_tensor("ident", [128, 128], BF16, kind="ExternalInput").ap()
    yT_out = nc.dram_tensor("yT", [D, 512], F32, kind="ExternalOutput").ap()
    import os
    DBG = os.environ.get("KDBG", "0") == "1"
    if DBG:
        dbg_out = nc.dram_tensor("dbg", [128, 12, 512], BF16,
                                 kind="ExternalOutput").ap()

    with tile.TileContext(nc) as tc:
        with (
            tc.tile_pool(name="persist", bufs=1) as persist,
            tc.tile_pool(name="ld", bufs=1) as ld,
            tc.tile_pool(name="stats", bufs=2) as stats,
            tc.tile_pool(name="dram", bufs=1, space="DRAM") as dram,
        ):
            # ---- constants / small inputs ----
            ones_c = persist.tile([128, 1], BF16, tag="ones_c")
            nc.vector.memset(ones_c, 1.0)
            rowinit = persist.tile([1, 128], F32, tag="rowinit")
            ones_rf = persist.tile([1, 128], F32R, tag="ones_rf")
            nc.vector.memset(rowinit, 1.0)
            with nc.allow_low_precision(reason="f32r constant rows"):
                nc.vector.tensor_copy(ones_rf, rowinit)
            ones_rb = persist.tile([1, 128], BF16, tag="ones_rb")
            nc.vector.memset(ones_rb, 1.0)
            rowinit2 = persist.tile([1, 128], F32, tag="rowinit2")
            lam_r = persist.tile([1, 128], F32R, tag="lam_r")
            nc.vector.memset(rowinit2, float(lam))
            with nc.allow_low_precision(reason="f32r constant rows"):
                nc.vector.tensor_copy(lam_r, rowinit2)
            eps128 = persist.tile([128, 1], F32, tag="eps128")
            nc.vector.memset(eps128, EPS)
            eps1 = persist.tile([1, 1], F32, tag="eps1")
            nc.vector.memset(eps1, EPS)
            subln_t = persist.tile([128, 1], F32, tag="subln")
            nc.sync.dma_start(out=subln_t,
                              in_=subln_eff.rearrange("(p one) -> p one", one=1))
            qb_t = persist.tile([128, 1], F32, tag="qb_t")
            nc.sync.dma_start(out=qb_t, in_=qb_in.rearrange("(p one) -> p one", one=1))
            kb_t = persist.tile([128, 1], F32, tag="kb_t")
            nc.sync.dma_start(out=kb_t, in_=kb_in.rearrange("(p one) -> p one", one=1))
            vb4_r = persist.tile([1, 512], BF16, tag="vb4_r")
            nc.sync.dma_start(out=vb4_r, in_=vb4_in.rearrange("(one f) -> one f", one=1))
            inb_t = persist.tile([128, 2 * NI], F32, tag="inb_t")
            nc.sync.dma_start(out=inb_t, in_=inb_in.rearrange("(k p) -> p k", p=128))

            # ---- fp8 weights + A2A landing (D/E lifetime; DMAs issued post-B) ----
            pW_cm = tc.tile_pool(name="pW", bufs=1)
            pW = pW_cm.__enter__()
            wo8 = pW.tile([128, DK // 2, 2, D], F8, tag="wo8")
            wgu8 = pW.tile([128, NI, 2, DK // 2, 2, 128], F8, tag="wgu8")
            wout8 = pW.tile([128, NI // 2, 2, D], F8, tag="wout8")
            af = [pW.tile([128, DK, 4, 64], F8, tag=f"af{b}", name=f"af{b}")
                  for b in range(B)]

            # ---- stage A/B/C lifetime pool ----
            pqkv_cm = tc.tile_pool(name="pqkv", bufs=1)
            pqkv = pqkv_cm.__enter__()
            qT = [pqkv.tile([128, 512], BF16, tag=f"qT{s}", name=f"qT{s}")
                  for s in range(NSIG)]
            kT = [pqkv.tile([128, 512], BF16, tag=f"kT{s}", name=f"kT{s}")
                  for s in range(NSIG)]
            v_t = [pqkv.tile([128, 512], BF16, tag=f"v{s}", name=f"v{s}")
                   for s in range(NSIG)]
            ident = pqkv.tile([128, 128], BF16, tag="ident")
            nc.sync.dma_start(out=ident, in_=ident_in)
            masks = pqkv.tile([128, 4, 512], BF16, tag="masks")
            nc.sync.dma_start(out=masks, in_=masks_in)
            wq_sb = pqkv.tile([128, D], BF16, tag="wq_sb")
            wk_sb = pqkv.tile([128, D], BF16, tag="wk_sb")
            wv_sb = pqkv.tile([128, D], BF16, tag="wv_sb")
            # qkv weights land before the bulk x stream
            for sb_t, wsrc in ((wq_sb, wq_s), (wk_sb, wk_s), (wv_sb, wv_s)):
                nc.sync.dma_start(
                    out=sb_t.rearrange("p (k m) -> p k m", m=128),
                    in_=wsrc.rearrange("(k p) m -> p k m", p=128))

            # AllToAll bounce buffers, one per (b, sl) sigma block: chunk u of
            # the input is o_fin[:, 64u:64u+64]; after the exchange, out[h] is
            # head h's o_fin for OUR 64-token unit of that sigma block.
            a2a_in = [dram.tile([N_CORES, 128, 2, 64], F8, tag=f"a2ai{g}",
                                name=f"a2ai{g}") for g in range(4)]
            a2a_out = [dram.tile([N_CORES, 128, 2, 64], F8, tag=f"a2ao{g}",
                                 name=f"a2ao{g}") for g in range(4)]

            # shared PSUM pool for stages A+B+C (8 banks exactly)
            psC_cm = tc.tile_pool(name="psC", bufs=1, space="PSUM")
            psC = psC_cm.__enter__()

            # v bias broadcast [128, 512] (4 repeats of the 128-wide bias row)
            pbv = psC.tile([128, 512], F32, tag="o1")
            nc.tensor.matmul(pbv, lhsT=ones_rb, rhs=vb4_r, start=True, stop=True)
            bv_bc = pqkv.tile([128, 512], F32, tag="bv_bc")
            nc.vector.tensor_copy(bv_bc, pbv)

            # ================= Stage A: LN1 + transpose =================
            # hT is a rotating per-sigma pipeline: stage B only reads the
            # tiles of its own sigma block, so 3 bufs per group suffice.
            phT_cm = tc.tile_pool(name="phT", bufs=1)
            phT = phT_cm.__enter__()
            hTs = []
            for s8 in range(NSIG):
                hTg = [phT.tile([128, 4, 512], BF16, tag=f"hT{g}", bufs=3,
                                name=f"hT{g}_{s8}") for g in range(2)]
                hTs.append(hTg)
                mvg = stats.tile([128, 4, 2], F32, tag="mvg")
                x4 = []
                for j4 in range(4):
                    st = s8 * 4 + j4
                    x_t = ld.tile([128, D], BF16, tag="x_t", bufs=6)
                    nc.sync.dma_start(out=x_t, in_=x_nat[st * 128:(st + 1) * 128, :])
                    st_t = stats.tile([128, 2, 6], F32, tag="bst")
                    xg = x_t.rearrange("p (g d) -> p g d", g=2)
                    for g in range(2):
                        nc.vector.bn_stats(out=st_t[:, g, :], in_=xg[:, g, :])
                    nc.vector.bn_aggr(out=mvg[:, j4, :], in_=st_t)
                    x4.append(x_t)
                rstd4 = stats.tile([128, 4], F32, tag="rstd4")
                nc.scalar.activation(out=rstd4, in_=mvg[:, :, 1], func=Sqrt,
                                     bias=eps128, scale=1.0)
                nc.vector.reciprocal(out=rstd4, in_=rstd4)
                nmr = stats.tile([128, 4], F32, tag="nmr")
                nc.vector.scalar_tensor_tensor(out=nmr, in0=mvg[:, :, 0],
                                               scalar=-1.0, in1=rstd4,
                                               op0=AluMult, op1=AluMult)
                for j4 in range(4):
                    st = s8 * 4 + j4
                    h_t = ld.tile([128, D], BF16, tag="h_t", bufs=4)
                    nc.scalar.activation(out=h_t, in_=x4[j4], func=Ident,
                                         scale=rstd4[:, j4:j4 + 1],
                                         bias=nmr[:, j4:j4 + 1])
                    jcol = slice(j4 * 128, (j4 + 1) * 128)
                    for g4 in range(2):
                        tpw = psC.tile([128, 1024], BF16, tag="tp", bufs=2, name="tp")
                        tp = tpw[:, 0:512]
                        for j in range(4):
                            dk = g4 * 4 + j
                            nc.tensor.transpose(tp[:, j * 128:(j + 1) * 128],
                                                h_t[:, dk * 128:(dk + 1) * 128], ident)
                        dst = hTs[s8][g4][:, :, jcol]
                        srcv = tp.rearrange("p (j f) -> p j f", f=128)
                        if (st + g4) % 2 == 0:
                            nc.vector.tensor_copy(dst, srcv)
                        else:
                            nc.scalar.copy(dst, srcv)

            # ================= Stage B: q,k,v projections =================
            for sg in range(NSIG):
                psq = psC.tile([128, 512], F32, tag="s12", bufs=2)
                for kk in range(DK):
                    nc.tensor.matmul(psq, lhsT=wq_sb[:, kk * 128:(kk + 1) * 128],
                                     rhs=hTs[sg][kk // 4][:, kk % 4, :],
                                     start=(kk == 0), stop=(kk == DK - 1))
                nc.scalar.activation(out=qT[sg], in_=psq, func=Ident,
                                     scale=1.0, bias=qb_t)
                psk = psC.tile([128, 512], F32, tag="s12", bufs=2)
                for kk in range(DK):
                    nc.tensor.matmul(psk, lhsT=wk_sb[:, kk * 128:(kk + 1) * 128],
                                     rhs=hTs[sg][kk // 4][:, kk % 4, :],
                                     start=(kk == 0), stop=(kk == DK - 1))
                nc.scalar.activation(out=kT[sg], in_=psk, func=Ident,
                                     scale=1.0, bias=kb_t)
                psv = psC.tile([128, 512], F32, tag="s12", bufs=2)
                for j4 in range(4):
                    for kk in range(DK):
                        nc.tensor.matmul(psv[:, j4 * 128:(j4 + 1) * 128],
                                         lhsT=hTs[sg][kk // 4][:, kk % 4, j4 * 128:(j4 + 1) * 128],
                                         rhs=wv_sb[:, kk * 128:(kk + 1) * 128],
                                         start=(kk == 0), stop=(kk == DK - 1))
                nc.vector.tensor_add(v_t[sg], psv, bv_bc)
            phT_cm.__exit__(None, None, None)

            # ---- weight prefetch (overlaps attention) ----
            nc.sync.dma_start(out=wo8, in_=wo8_in)
            nc.sync.dma_start(out=wgu8, in_=wgu8_in)
            nc.sync.dma_start(out=wout8, in_=wout8_in)

            # ================= Stage C: differential attention =================
            pwc_cm = tc.tile_pool(name="pwc", bufs=1)
            pwc = pwc_cm.__enter__()
            for b in range(B):
                for sl in (0, 1, 2, 3):
                    sg = 4 * b + sl
                    ntau = 4 * (sl + 1)
                    o1 = psC.tile([128, 512], F32, tag="o1")
                    o2 = psC.tile([128, 512], F32, tag="o2")
                    esum1 = pwc.tile([128, 512], BF16, tag="es1", bufs=2)
                    esum2 = pwc.tile([128, 512], BF16, tag="es2", bufs=2)
                    for tau in range(ntau):
                        tg = 16 * b + tau
                        ts8, tj = tg // 4, tg % 4
                        tcol = slice(tj * 128, (tj + 1) * 128)
                        rel = tau - 4 * sl
                        off = max(rel, 0) * 128          # causal column offset
                        ecol = slice(off, 512)
                        st_fl = (tau == 0)
                        sp_fl = (tau == ntau - 1)
                        s12 = psC.tile([128, 2, 512], F32, tag="s12", bufs=2)
                        nc.tensor.matmul(s12[:, 0, ecol], lhsT=kT[ts8][0:64, tcol],
                                         rhs=qT[sg][0:64, ecol], start=True, stop=True)
                        nc.tensor.matmul(s12[:, 1, ecol], lhsT=kT[ts8][64:128, tcol],
                                         rhs=qT[sg][64:128, ecol], start=True, stop=True)
                        e12 = pwc.tile([128, 2, 512], BF16, tag="e12", bufs=6)
                        nc.scalar.activation(out=e12[:, :, ecol], in_=s12[:, :, ecol],
                                             func=Exp)
                        e1 = e12[:, 0, :]
                        e2 = e12[:, 1, :]
                        if rel >= 0:
                            # only the 128-wide diagonal strip needs masking
                            strip = slice(off, off + 128)
                            tri = masks[:, 0, 0:128]
                            nc.gpsimd.tensor_mul(e1[:, strip], e1[:, strip], tri)
                            nc.vector.tensor_mul(e2[:, strip], e2[:, strip], tri)
                        if st_fl:
                            nc.gpsimd.tensor_copy(esum1, e1)
                            nc.vector.tensor_copy(esum2, e2)
                        else:
                            nc.gpsimd.tensor_add(esum1[:, ecol], e1[:, ecol],
                                                 esum1[:, ecol])
                            nc.vector.tensor_add(esum2[:, ecol], e2[:, ecol],
                                                 esum2[:, ecol])
                        nc.tensor.matmul(o1[:, ecol], lhsT=v_t[ts8][:, tcol],
                                         rhs=e1[:, ecol], start=st_fl, stop=sp_fl)
                        nc.tensor.matmul(o2[:, ecol], lhsT=v_t[ts8][:, tcol],
                                         rhs=e2[:, ecol], start=st_fl, stop=sp_fl)
                    # ---- differential combine + subln ----
                    z1 = psC.tile([1, 512], F32, tag="tp", bufs=2)
                    nc.tensor.matmul(z1, lhsT=ones_c, rhs=esum1, start=True, stop=True)
                    z2 = psC.tile([1, 512], F32, tag="tp", bufs=2)
                    nc.tensor.matmul(z2, lhsT=ones_c, rhs=esum2, start=True, stop=True)
                    zrec = stats.tile([1, 512], F32, tag="zrec")
                    nc.vector.reciprocal_approx_fast(out=zrec, in_=z2)
                    zr = stats.tile([1, 512], F32R, tag="rowf1")
                    with nc.allow_low_precision(reason="softmax ratio to f32r row"):
                        nc.vector.tensor_mul(zr, z1, zrec)
                    w_bc = psC.tile([128, 512], F32, tag="tp", bufs=2)
                    nc.tensor.matmul(w_bc, lhsT=lam_r, rhs=zr, start=True, stop=True)
                    w_sb = pwc.tile([128, 512], F32, tag="w_sb")
                    nc.vector.tensor_copy(w_sb, w_bc)
                    t_sb = pwc.tile([128, 512], F32, tag="t_sb")
                    nc.vector.tensor_mul(t_sb, o2, w_sb)
                    oc = pwc.tile([128, 512], F32, tag="oc")
                    nc.vector.tensor_sub(oc, o1, t_sb)
                    sq = pwc.tile([128, 512], BF16, tag="sq")
                    nc.gpsimd.tensor_mul(sq, oc, oc)
                    ss = psC.tile([1, 512], F32, tag="tp", bufs=2)
                    nc.tensor.matmul(ss, lhsT=ones_c, rhs=sq, start=True, stop=True)
                    rt = stats.tile([1, 512], F32, tag="rt")
                    nc.scalar.activation(out=rt, in_=ss, func=Sqrt,
                                         scale=1.0 / 128.0, bias=eps1)
                    rrf = stats.tile([1, 512], F32, tag="rowf3")
                    nc.vector.reciprocal_approx_fast(out=rrf, in_=rt)
                    rr = stats.tile([1, 512], F32R, tag="rowf2")
                    with nc.allow_low_precision(reason="rms recip to f32r row"):
                        nc.vector.tensor_copy(rr, rrf)
                    r_bc = psC.tile([128, 512], F32, tag="tp", bufs=2)
                    nc.tensor.matmul(r_bc, lhsT=ones_rf, rhs=rr, start=True, stop=True)
                    t2 = pwc.tile([128, 512], F32, tag="t2")
                    nc.vector.tensor_mul(t2, oc, r_bc)
                    o_fin = pwc.tile([128, 512], F8, tag="o_fin", bufs=4)
                    nc.vector.tensor_scalar_mul(o_fin, t2, subln_t)
                    grp = 2 * b + sl // 2
                    nc.sync.dma_start(
                        out=a2a_in[grp][:, :, sl % 2, :].rearrange("u p f -> p u f"),
                        in_=o_fin.rearrange("p (u f) -> p u f", f=64))
                    if sl % 2 == 1:
                        # AllToAll per sigma pair: all but the last overlap
                        # the remaining attention compute
                        nc.gpsimd.collective_compute(
                            "AllToAll", mybir.AluOpType.bypass, replica_groups=RG,
                            ins=[a2a_in[grp].opt()], outs=[a2a_out[grp].opt()])
                if DBG and b == 0:
                    dv = dbg_out.rearrange("p s f -> p (s f)")
                    nc.sync.dma_start(
                        out=dv[:, 0:2048].rearrange("p (u f) -> u p f", f=256),
                        in_=a2a_in[0])
                    nc.sync.dma_start(
                        out=dv[:, 2048:4096].rearrange("p (u f) -> u p f", f=256),
                        in_=a2a_out[0])
                    nc.sync.dma_start(
                        out=dv[:, 4096:6144].rearrange("p (u f) -> p u f", f=256),
                        in_=af[0])
            pwc_cm.__exit__(None, None, None)
            psC_cm.__exit__(None, None, None)
            pqkv_cm.__exit__(None, None, None)

            # ================= Stages D+E, split by batch =================
            # b=0's collectives land mid-attention, so its wo + LN2 + FFN
            # half hides under the final (b=1) AllToAll latency.
            pE_cm = tc.tile_pool(name="pE", bufs=1)
            pE = pE_cm.__enter__()
            psE_cm = tc.tile_pool(name="psE", bufs=1, space="PSUM")
            psE = psE_cm.__enter__()
            y1own = [pE.tile([128, 512], F32, tag=f"y1own{dk}", name=f"y1own{dk}")
                     for dk in range(DK)]
            y1bf = [pE.tile([128, 512], BF16, tag=f"y1bf{dk}", name=f"y1bf{dk}")
                    for dk in range(DK)]
            h2p = [pE.tile([128, 2, 512], F8, tag=f"h2p{k}", name=f"h2p{k}")
                   for k in range(DK // 2)]
            su_p = [pE.tile([128, 2, 512], F8, tag=f"sup{k}", name=f"sup{k}")
                    for k in range(NI // 2)]
            for b in range(B):
                cb = slice(b * 256, (b + 1) * 256)
                for sl in range(4):
                    nc.sync.dma_start(
                        out=af[b][:, :, sl, :],
                        in_=a2a_out[2 * b + sl // 2][:, :, sl % 2, :]
                        .rearrange("h p f -> p h f"))
                # ---- wo + residual for this batch's 256 tokens ----
                for dm in range(DK):
                    dmc = slice(dm * 128, (dm + 1) * 128)
                    xo_t = ld.tile([128, 256], F32, tag="xo_t", bufs=2)
                    nc.sync.dma_start(out=xo_t, in_=xT_own[dmc, cb])
                    pwo = psE.tile([128, 256], F32, tag="ey2", bufs=2)
                    afp = af[b].rearrange("p (h2 i) s f -> p h2 i (s f)", i=2)
                    for h2 in range(DK // 2):
                        nc.tensor.matmul(pwo, lhsT=wo8[:, h2, :, dmc],
                                         rhs=afp[:, h2], start=(h2 == 0),
                                         stop=(h2 == DK // 2 - 1), perf_mode=DR)
                    nc.vector.scalar_tensor_tensor(
                        out=y1own[dm][:, cb], in0=pwo, scalar=1.0 / (SW * SO),
                        in1=xo_t, op0=AluMult, op1=AluAdd)
                    with nc.allow_low_precision(reason="ffn input is bf16"):
                        nc.vector.scalar_tensor_tensor(
                            out=y1bf[dm][:, cb], in0=pwo, scalar=1.0 / (SW * SO),
                            in1=xo_t, op0=AluMult, op1=AluAdd)
                # ---- LN2 for this batch half ----
                ssum = psE.tile([1, 256], F32, tag="es")
                ssq = psE.tile([1, 256], F32, tag="esq")
                for dk in range(DK):
                    nc.tensor.matmul(ssum, lhsT=ones_c, rhs=y1bf[dk][:, cb],
                                     start=(dk == 0), stop=(dk == DK - 1))
                    sqt = ld.tile([128, 256], BF16, tag="sqt", bufs=2)
                    nc.vector.tensor_mul(sqt, y1bf[dk][:, cb], y1bf[dk][:, cb])
                    nc.tensor.matmul(ssq, lhsT=ones_c, rhs=sqt,
                                     start=(dk == 0), stop=(dk == DK - 1))
                m_row = stats.tile([1, 256], F32, tag="rowf1")
                nc.vector.tensor_scalar_mul(m_row, ssum, 1.0 / float(D))
                mm_row = stats.tile([1, 256], F32, tag="rowf2")
                nc.vector.tensor_mul(mm_row, m_row, m_row)
                v_row = stats.tile([1, 256], F32, tag="rowf3")
                nc.vector.tensor_scalar_mul(v_row, ssq, 1.0 / float(D))
                nc.vector.tensor_sub(v_row, v_row, mm_row)
                nc.scalar.activation(out=v_row, in_=v_row, func=Sqrt,
                                     scale=1.0, bias=eps1)
                r_row = stats.tile([1, 256], F32R, tag="rowf4")
                mr_row = stats.tile([1, 256], F32R, tag="rowf5")
                with nc.allow_low_precision(reason="ln2 rows to f32r"):
                    nc.vector.reciprocal(out=r_row, in_=v_row)
                    nc.vector.tensor_mul(mr_row, m_row, r_row)
                pbc = psE.tile([128, 256], F32, tag="es")
                nc.tensor.matmul(pbc, lhsT=ones_rf, rhs=r_row, start=True, stop=True)
                rbc2 = pE.tile([128, 256], BF16, tag="rbc2", bufs=2)
                nc.vector.tensor_copy(rbc2, pbc)
                pbc2 = psE.tile([128, 256], F32, tag="esq")
                nc.tensor.matmul(pbc2, lhsT=ones_rf, rhs=mr_row, start=True, stop=True)
                mrbc = pE.tile([128, 256], BF16, tag="mrbc", bufs=2)
                nc.vector.tensor_copy(mrbc, pbc2)
                for dk in range(DK):
                    a = ld.tile([128, 256], BF16, tag="h2t", bufs=2)
                    nc.vector.tensor_mul(a, y1bf[dk][:, cb], rbc2)
                    nc.gpsimd.tensor_sub(h2p[dk // 2][:, dk % 2, cb], a, mrbc)
                # ---- FFN in-projection for this batch half ----
                for m in range(NI):
                    psg = psE.tile([128, 256], F32, tag="eg", bufs=2)
                    for kk in range(DK // 2):
                        nc.tensor.matmul(psg, lhsT=wgu8[:, m, 0, kk, :, :],
                                         rhs=h2p[kk][:, :, cb], start=(kk == 0),
                                         stop=(kk == DK // 2 - 1), perf_mode=DR)
                    psu = psE.tile([128, 256], F32, tag="eu", bufs=2)
                    for kk in range(DK // 2):
                        nc.tensor.matmul(psu, lhsT=wgu8[:, m, 1, kk, :, :],
                                         rhs=h2p[kk][:, :, cb], start=(kk == 0),
                                         stop=(kk == DK // 2 - 1), perf_mode=DR)
                    sg_t = pE.tile([128, 256], BF16, tag="sg_t", bufs=2)
                    nc.scalar.activation(out=sg_t, in_=psg, func=Silu,
                                         scale=1.0 / SW, bias=inb_t[:, m:m + 1])
                    tu = pE.tile([128, 256], F32, tag="tu", bufs=2)
                    nc.vector.tensor_scalar(out=tu, in0=psu, scalar1=SU / SW,
                                            scalar2=inb_t[:, NI + m:NI + m + 1],
                                            op0=AluMult, op1=AluAdd)
                    nc.vector.tensor_mul(su_p[m // 2][:, m % 2, cb], tu, sg_t)
                # ---- w_out + final residual for this batch half ----
                for dm in range(DK):
                    dmc = slice(dm * 128, (dm + 1) * 128)
                    py2 = psE.tile([128, 256], F32, tag="ey2", bufs=2)
                    for k in range(NI // 2):
                        nc.tensor.matmul(py2, lhsT=wout8[:, k, :, dmc],
                                         rhs=su_p[k][:, :, cb], start=(k == 0),
                                         stop=(k == NI // 2 - 1), perf_mode=DR)
                    yout = ld.tile([128, 256], F32, tag="yout", bufs=2)
                    nc.vector.scalar_tensor_tensor(
                        out=yout, in0=py2, scalar=1.0 / (SW * SU),
                        in1=y1own[dm][:, cb], op0=AluMult, op1=AluAdd)
                    nc.sync.dma_start(out=yT_out[dmc, cb], in_=yout)
            psE_cm.__exit__(None, None, None)
            pE_cm.__exit__(None, None, None)
            pW_cm.__exit__(None, None, None)

    nc.compile()
    return nc


def _to_f8(a):
    return np.clip(a, -440.0, 440.0).astype(NP_F8)


def _prep_inputs(inputs):
    """Host-side shard prep: returns (lam, in_maps)."""
    f = {k: np.asarray(v, dtype=np.float32) for k, v in inputs.items()}
    lam = float(np.exp(np.sum(f["lq1"] * f["lk1"]))
                - np.exp(np.sum(f["lq2"] * f["lk2"])) + LAMBDA_INIT)
    x = f["x"].reshape(NS, D)
    x_bf = x.astype(NP_BF16)
    xT = np.ascontiguousarray(x.T)                       # [D, NS]
    # causal masks [pt, rel, cs]: allowed iff pt <= cs - 128*rel
    pt = np.arange(128)[:, None, None]
    rl = np.arange(4)[None, :, None]
    cs = np.arange(512)[None, None, :]
    masks = (pt <= cs - 128 * rl).astype(NP_BF16)
    ident = np.eye(128, dtype=NP_BF16)
    subln_base = (f["subln_w"] * (1.0 - LAMBDA_INIT) * SO).astype(np.float32)
    s8 = float(HD) ** -0.5
    l1w = f["ln1_w"][:, None]
    wq_e = l1w * f["wq"] * s8
    wk_e = l1w * f["wk"]
    wv_e = l1w * f["wv"]
    qb_full = f["ln1_b"] @ f["wq"] * s8                  # [D]
    kb_full = f["ln1_b"] @ f["wk"]
    vb_full = f["ln1_b"] @ f["wv"]
    w_in_e = f["ln2_w"][:, None] * f["w_in"]             # [D, 2*FFN] f32
    inb = (f["ln2_b"] @ f["w_in"]).astype(np.float32)    # [2*FFN]
    inb_sc = inb.copy()
    inb_sc[FFN:] *= SU                                   # up-bias pre-scaled
    # fp8 DoubleRow weight layouts (pre-scaled by SW)
    # wo8[p, h2, i, m] = wo[(2*h2+i)*128 + p, m] * SW
    wo8 = _to_f8((f["wo"] * SW).reshape(DK // 2, 2, 128, D)
                 .transpose(2, 0, 1, 3))
    # wgu8[p, m, g, kk0, i, c] = w_in_e[(2*kk0+i)*128+p, g*FFN + m*128+c] * SW
    wgu = (w_in_e * SW).reshape(DK // 2, 2, 128, 2, NI, 128)
    wgu8 = _to_f8(np.ascontiguousarray(wgu.transpose(2, 4, 3, 0, 1, 5)))
    # wout8[p, kk0, i, m] = w_out[(2*kk0+i)*128+p, m] * SW
    wout8 = _to_f8((f["w_out"] * SW).reshape(NI // 2, 2, 128, D)
                   .transpose(2, 0, 1, 3))
    in_maps = []
    for c in range(N_CORES):
        hc = slice(128 * c, 128 * (c + 1))
        # core c owns tokens [64c, 64c+64) of each (batch, sl) sigma block
        xo = np.concatenate(
            [xT[:, b * S + 512 * sl + 64 * c: b * S + 512 * sl + 64 * c + 64]
             for b in range(B) for sl in range(4)], axis=1)
        in_maps.append({
            "x_nat": x_bf,
            "xT_own": np.ascontiguousarray(xo),
            "wq_s": wq_e[:, hc].astype(NP_BF16),
            "wk_s": wk_e[:, hc].astype(NP_BF16),
            "wv_s": wv_e[:, hc].astype(NP_BF16),
            "wo8": wo8,
            "wgu8": wgu8,
            "wout8": wout8,
            "qb": np.ascontiguousarray(qb_full[hc]),
            "kb": np.ascontiguousarray(kb_full[hc]),
            "vb4": np.tile(vb_full[hc], 4).astype(NP_BF16),
            "inb": inb_sc,
            "subln_eff": subln_base,
            "masks": masks, "ident": ident,
        })
    return lam, in_maps


_CACHE = {}


def _run(inputs, trace=False, trace_kwargs=None):
    lam, in_maps = _prep_inputs(inputs)
    key = round(lam, 10)
    if key not in _CACHE:
        _CACHE[key] = build_program(lam)
    nc = _CACHE[key]
    res = bass_utils.run_bass_kernel_spmd(
        nc, in_maps, core_ids=list(range(N_CORES)),
        trace=trace, **(trace_kwargs or {}))
    y = np.empty((NS, D), dtype=np.float32)
    for c in range(N_CORES):
        yT = res.results[c]["yT"]                        # [D, 512]
        for b in range(B):
            for sl in range(4):
                fb = b * S + 512 * sl + 64 * c
                cb = (4 * b + sl) * 64
                y[fb:fb + 64, :] = yT[:, cb:cb + 64].T
    return y.reshape(B, S, D), res


def kernel(**inputs) -> np.ndarray:
    y, _ = _run(inputs)
    return y
